# revision 12
# baseline (speedup 1.0000x reference)
"""Trainium2 Bass kernel for nn_AxialBottleneck (conv1x1+BN+relu -> axial-attn(H)
-> axial-attn(W) -> relu -> conv1x1+BN -> relu(+residual)).

Self-contained: accepts FULL inputs, shards across 8 NeuronCores internally
(axial-H sharded over W, axial-W sharded over H; two SPMD launches with a host
reshard between), returns the FULL output.
"""

import sys
from contextlib import ExitStack

import numpy as np

for _p in ("/opt/trn_rl_repo",):
    if _p not in sys.path:
        sys.path.insert(0, _p)

import concourse.bass as bass
import concourse.bacc as bacc
import concourse.mybir as mybir
import concourse.tile as tile
from concourse.bass_utils import run_bass_kernel_spmd

EPS = 1e-5
HEADS = 8
NCORES = 8
F32 = mybir.dt.float32
AF = mybir.ActivationFunctionType
ALU = mybir.AluOpType

# channel permutation: old kqv row 16h+i -> new row (k: 8i+h | q: 32+8(i-4)+h | v: 64+8(i-8)+h)
def _old_of_new():
    o = np.zeros(128, np.int64)
    for old in range(128):
        h, i = old // 16, old % 16
        if i < 4:
            new = 8 * i + h
        elif i < 8:
            new = 32 + 8 * (i - 4) + h
        else:
            new = 64 + 8 * (i - 8) + h
        o[new] = old
    return o


def _bn_sb(p):
    g, b, m, v = p.astype(np.float64)
    s = g / np.sqrt(v + EPS)
    return (s).astype(np.float32), (b - m * s).astype(np.float32)


def _enc_arrays(renc, L):
    # o[a, b] = b - a + L - 1
    o = np.arange(L)[None, :] - np.arange(L)[:, None] + L - 1
    qencS = renc[0:4][:, o]   # [4, x, y] = renc[d, y-x+L-1]
    kencST = renc[4:8][:, o]  # [4, y, x] = renc[4+d, x-y+L-1]
    vencS = renc[8:16][:, o]  # [8, x, y] = renc[8+d, y-x+L-1]
    return (np.ascontiguousarray(qencS, np.float32),
            np.ascontiguousarray(kencST, np.float32),
            np.ascontiguousarray(vencS, np.float32))


def _axial_folds(kqv_w, kqv_bn, logits_bn):
    """Permuted lhsT weight [64,128] (cols: k per-head 4h.., q 32+4h.., v 64+8h+d),
    per-head ACT scale/bias [4,8] for k and q, v scale/bias [64,1], s2[8], s31[8]."""
    sBN, bBN = _bn_sb(kqv_bn)          # [128]
    sL, _ = _bn_sb(logits_bn)          # [24]; bias part drops in softmax
    s1, s2, s3 = sL[0:8], sL[8:16], sL[16:24]
    old_k = np.array([16 * h + d for h in range(HEADS) for d in range(4)])
    old_q = np.array([16 * h + 4 + d for h in range(HEADS) for d in range(4)])
    old_v = np.array([16 * h + 8 + d for h in range(HEADS) for d in range(8)])
    order = np.concatenate([old_k, old_q, old_v])
    Wp = kqv_w[order]                  # [128, 64]
    ks4 = (sBN[old_k] * np.repeat(s1, 4)).reshape(8, 4).T.copy()   # [4, 8]
    kb4 = (bBN[old_k] * np.repeat(s1, 4)).reshape(8, 4).T.copy()
    qs4 = sBN[old_q].reshape(8, 4).T.copy()
    qb4 = bBN[old_q].reshape(8, 4).T.copy()
    vs = sBN[old_v].reshape(64, 1)
    vb = bBN[old_v].reshape(64, 1)
    return (np.ascontiguousarray(Wp.T, np.float32),
            ks4.astype(np.float32), kb4.astype(np.float32),
            qs4.astype(np.float32), qb4.astype(np.float32),
            vs.astype(np.float32), vb.astype(np.float32),
            s2.astype(np.float32), (s3 / s1).astype(np.float32))


def _build_axial(nc, L, n_per, first, s2, s31):
    """first=True: conv1 + axial-H (xin [64, 256*wsh] W-sliced; out oHT
    [n_per, 256, 64]). first=False: axial-W + relu + conv3 + bn3 + residual
    relu (xin/res [64, n_per*128] H-sliced; out outn [64, n_per*128])."""
    nyt = L // 128
    nxt = L // 128
    if first:
        wsh = 128 // NCORES
        xin = nc.dram_tensor("xin", [64, 256 * wsh], F32, kind="ExternalInput")
        w1t = nc.dram_tensor("w1t", [64, 64], F32, kind="ExternalInput")
        c1s = nc.dram_tensor("c1s", [64, 1], F32, kind="ExternalInput")
        c1b = nc.dram_tensor("c1b", [64, 1], F32, kind="ExternalInput")
        oHT = nc.dram_tensor("oHT", [n_per, 256, 64], F32, kind="ExternalOutput")
    else:
        xin = nc.dram_tensor("xin", [64, n_per * 128], F32, kind="ExternalInput")
        res = nc.dram_tensor("res", [64, n_per * 128], F32, kind="ExternalInput")
        w3t = nc.dram_tensor("w3t", [64, 64], F32, kind="ExternalInput")
        c3s = nc.dram_tensor("c3s", [64, 1], F32, kind="ExternalInput")
        c3b = nc.dram_tensor("c3b", [64, 1], F32, kind="ExternalInput")
        outn = nc.dram_tensor("outn", [64, n_per * 128], F32, kind="ExternalOutput")
    wkqvt = nc.dram_tensor("wkqvt", [64, 128], F32, kind="ExternalInput")
    ks4 = nc.dram_tensor("ks4", [4, 8], F32, kind="ExternalInput")
    kb4 = nc.dram_tensor("kb4", [4, 8], F32, kind="ExternalInput")
    qs4 = nc.dram_tensor("qs4", [4, 8], F32, kind="ExternalInput")
    qb4 = nc.dram_tensor("qb4", [4, 8], F32, kind="ExternalInput")
    vsc = nc.dram_tensor("vsc", [64, 1], F32, kind="ExternalInput")
    vbc = nc.dram_tensor("vbc", [64, 1], F32, kind="ExternalInput")
    qenc = nc.dram_tensor("qenc", [4, L, L], F32, kind="ExternalInput")
    kenc = nc.dram_tensor("kenc", [4, L, L], F32, kind="ExternalInput")
    venc = nc.dram_tensor("venc", [8, L, L], F32, kind="ExternalInput")
    ident = nc.dram_tensor("ident", [128, 128], F32, kind="ExternalInput")

    with ExitStack() as ctx:
        tc = ctx.enter_context(tile.TileContext(nc))
        singles = ctx.enter_context(tc.tile_pool(name="singles", bufs=1))
        work = ctx.enter_context(tc.tile_pool(name="work", bufs=3))
        wide = ctx.enter_context(tc.tile_pool(name="wide", bufs=2))
        ps_mm = ctx.enter_context(tc.tile_pool(name="ps_mm", bufs=2, space="PSUM"))
        ps_lt = ctx.enter_context(tc.tile_pool(name="ps_lt", bufs=2, space="PSUM"))
        ps_el = ctx.enter_context(tc.tile_pool(name="ps_el", bufs=1, space="PSUM"))
        ps_at = ctx.enter_context(tc.tile_pool(name="ps_at", bufs=1, space="PSUM"))
        ps_tp = ctx.enter_context(tc.tile_pool(name="ps_tp", bufs=2, space="PSUM"))

        def load(name, dram, shape):
            t = singles.tile(shape, F32, name=name, tag=name)
            nc.sync.dma_start(out=t, in_=dram[:, :] if len(shape) == 2 else dram)
            return t

        ident_sb = load("ident", ident, [128, 128])
        wkqvt_d = load("wkqvt", wkqvt, [64, 128])
        wkqvt_sb = singles.tile([64, 128], F32, name="wkqvt_c", tag="wkqvt_c")
        nc.vector.tensor_scalar(wkqvt_sb, wkqvt_d, 1.0, None, ALU.mult)
        ks4_sb = load("ks4", ks4, [4, 8])
        kb4_sb = load("kb4", kb4, [4, 8])
        qs4_sb = load("qs4", qs4, [4, 8])
        qb4_sb = load("qb4", qb4, [4, 8])
        vsc_sb = load("vsc", vsc, [64, 1])
        vbc_sb = load("vbc", vbc, [64, 1])
        qenc_sb = {}
        kenc_sb = {}
        venc_sb = {}
        for d in range(4):
            for xt in range(nxt):
                t = singles.tile([128, L], F32, name=f"qe{d}_{xt}", tag=f"qe{d}_{xt}")
                nc.sync.dma_start(out=t, in_=qenc[d, xt * 128:(xt + 1) * 128, :])
                qenc_sb[(d, xt)] = t
            for yt in range(nyt):
                t = singles.tile([128, L], F32, name=f"ke{d}_{yt}", tag=f"ke{d}_{yt}")
                nc.sync.dma_start(out=t, in_=kenc[d, yt * 128:(yt + 1) * 128, :])
                kenc_sb[(d, yt)] = t
        for d in range(8):
            for xt in range(nxt):
                t = singles.tile([128, L], F32, name=f"ve{d}_{xt}", tag=f"ve{d}_{xt}")
                nc.sync.dma_start(out=t, in_=venc[d, xt * 128:(xt + 1) * 128, :])
                venc_sb[(d, xt)] = t

        if first:
            w1t_d = load("w1t", w1t, [64, 64])
            w1t_sb = singles.tile([64, 64], F32, name="w1t_c", tag="w1t_c")
            nc.vector.tensor_scalar(w1t_sb, w1t_d, 1.0, None, ALU.mult)
            c1s_sb = load("c1s", c1s, [64, 1])
            c1b_sb = load("c1b", c1b, [64, 1])
            wsh = 128 // NCORES
            npx = 256 * wsh
            xin_sb = load("xin", xin, [64, npx])
            conv_sb = singles.tile([64, npx], F32, name="conv", tag="conv")
            for ck in range(npx // 512):
                cp = ps_mm.tile([64, 512], F32, name="mmp", tag="mmp")
                nc.tensor.matmul(cp, w1t_sb, xin_sb[:, ck * 512:(ck + 1) * 512],
                                 start=True, stop=True)
                nc.scalar.activation(conv_sb[:, ck * 512:(ck + 1) * 512], cp,
                                     AF.Relu, bias=c1b_sb[:, 0:1], scale=c1s_sb[:, 0:1])
        else:
            w3t_d = load("w3t", w3t, [64, 64])
            w3t_sb = singles.tile([64, 64], F32, name="w3t_c", tag="w3t_c")
            nc.vector.tensor_scalar(w3t_sb, w3t_d, 1.0, None, ALU.mult)
            c3s_sb = load("c3s", c3s, [64, 1])
            c3b_sb = load("c3b", c3b, [64, 1])
            xin_sb = load("xin", xin, [64, n_per * 128])
            res_sb = load("res", res, [64, n_per * 128])

        for n in range(n_per):
            if first:
                wsh = 128 // NCORES
                xf = conv_sb[:, n:256 * wsh:wsh]
            else:
                xf = xin_sb[:, n * 128:(n + 1) * 128]
            # per-head k, q tiles [4, L]; v tile [64, L]
            kh, qh = [], []
            for h in range(HEADS):
                kp = ps_mm.tile([4, L], F32, name="mmp", tag="mmp")
                nc.tensor.matmul(kp, wkqvt_sb[:, 4 * h:4 * h + 4], xf,
                                 start=True, stop=True)
                kt = work.tile([4, L], F32, name="kh", tag="kh", bufs=10)
                nc.scalar.activation(kt, kp, AF.Identity,
                                     bias=kb4_sb[:, h:h + 1], scale=ks4_sb[:, h:h + 1])
                kh.append(kt)
                qp = ps_mm.tile([4, L], F32, name="mmp", tag="mmp")
                nc.tensor.matmul(qp, wkqvt_sb[:, 32 + 4 * h:32 + 4 * h + 4], xf,
                                 start=True, stop=True)
                qt = work.tile([4, L], F32, name="qh", tag="qh", bufs=10)
                nc.scalar.activation(qt, qp, AF.Identity,
                                     bias=qb4_sb[:, h:h + 1], scale=qs4_sb[:, h:h + 1])
                qh.append(qt)
            vp = ps_mm.tile([64, L], F32, name="mmp", tag="mmp")
            nc.tensor.matmul(vp, wkqvt_sb[:, 64:128], xf, start=True, stop=True)
            vt = wide.tile([64, L], F32, name="vt", tag="vt", bufs=2)
            nc.scalar.activation(vt, vp, AF.Identity,
                                 bias=vbc_sb[:, 0:1], scale=vsc_sb[:, 0:1])
            vT = []
            for yt in range(nyt):
                tp = ps_tp.tile([128, 64], F32, name="tpp", tag="tpp")
                nc.tensor.matmul(tp, vt[:, yt * 128:(yt + 1) * 128],
                                 ident_sb[0:64, 0:64], is_transpose=True,
                                 start=True, stop=True)
                ts = work.tile([128, 64], F32, name="vT", tag="vT", bufs=4)
                nc.scalar.activation(ts, tp, AF.Copy)
                vT.append(ts)
            aoT = []
            for xt in range(nxt):
                aoT.append(work.tile([128, 64], F32, name=f"aoT{xt}", tag=f"aoT{xt}"))
            for h in range(HEADS):
                ksc, qsc = [], []
                for yt in range(nyt):
                    tp = ps_tp.tile([128, 4], F32, name="tpp", tag="tpp")
                    nc.tensor.matmul(tp, kh[h][:, yt * 128:(yt + 1) * 128],
                                     ident_sb[0:4, 0:4], is_transpose=True,
                                     start=True, stop=True)
                    t = work.tile([128, 4], F32, name="ksc", tag="ksc", bufs=4)
                    nc.scalar.activation(t, tp, AF.Copy, scale=float(s31[h]))
                    ksc.append(t)
                for xt in range(nxt):
                    tp = ps_tp.tile([128, 4], F32, name="tpp", tag="tpp")
                    nc.tensor.matmul(tp, qh[h][:, xt * 128:(xt + 1) * 128],
                                     ident_sb[0:4, 0:4], is_transpose=True,
                                     start=True, stop=True)
                    t = work.tile([128, 4], F32, name="qsc", tag="qsc", bufs=4)
                    nc.scalar.activation(t, tp, AF.Copy, scale=float(s2[h]))
                    qsc.append(t)
                # qr natural [x, y]
                qr = []
                for xt in range(nxt):
                    t = wide.tile([128, L], F32, name="qr", tag="qr")
                    nc.vector.tensor_scalar(t, qenc_sb[(0, xt)], qsc[xt][:, 0:1],
                                            None, ALU.mult)
                    for d in range(1, 4):
                        nc.vector.scalar_tensor_tensor(
                            t, qenc_sb[(d, xt)], qsc[xt][:, d:d + 1], t,
                            ALU.mult, ALU.add)
                    qr.append(t)
                # logits^T [y, x] in PSUM: qk + qr^T + kr
                LT = []
                for yt in range(nyt):
                    lt = ps_lt.tile([128, L], F32, name="lt", tag="lt")
                    nc.tensor.matmul(lt, kh[h][:, yt * 128:(yt + 1) * 128],
                                     qh[h], start=True, stop=False)
                    for xt in range(nxt):
                        nc.tensor.matmul(lt[:, xt * 128:(xt + 1) * 128],
                                         qr[xt][:, yt * 128:(yt + 1) * 128],
                                         ident_sb, is_transpose=True,
                                         start=False, stop=(xt == nxt - 1))
                    for d in range(4):
                        nc.vector.scalar_tensor_tensor(
                            lt, kenc_sb[(d, yt)], ksc[yt][:, d:d + 1], lt,
                            ALU.mult, ALU.add)
                    LT.append(lt)
                expLT = []
                for yt in range(nyt):
                    e = wide.tile([128, L], F32, name="explt", tag="explt", bufs=4)
                    nc.scalar.activation(e, LT[yt], AF.Exp)
                    expLT.append(e)
                arhs = []
                for yt in range(nyt):
                    r = work.tile([128, 9], F32, name="arhs", tag="arhs", bufs=4)
                    nc.vector.tensor_scalar(r[:, 0:8], vT[yt][:, 8 * h:8 * h + 8],
                                            1.0, None, ALU.mult)
                    nc.vector.memset(r[:, 8:9], 1.0)
                    arhs.append(r)
                at = ps_at.tile([128, 9 * nxt], F32, name="at", tag="at")
                for xh in range(nxt):
                    for yt in range(nyt):
                        nc.tensor.matmul(at[:, xh * 9:xh * 9 + 9],
                                         expLT[yt][:, xh * 128:(xh + 1) * 128],
                                         arhs[yt], start=(yt == 0),
                                         stop=(yt == nyt - 1))
                for xh in range(nxt):
                    ep = ps_el.tile([128, L], F32, name="expl", tag="expl")
                    for yt in range(nyt):
                        nc.tensor.matmul(ep[:, yt * 128:(yt + 1) * 128],
                                         expLT[yt][:, xh * 128:(xh + 1) * 128],
                                         ident_sb, is_transpose=True,
                                         start=True, stop=True)
                    aenc = work.tile([128, 8], F32, name="aenc", tag="aenc")
                    scratch = wide.tile([128, L], F32, name="scratch", tag="scratch")
                    for d in range(8):
                        nc.vector.scalar_tensor_tensor(
                            scratch, venc_sb[(d, xh)], 1.0, ep,
                            ALU.mult, ALU.mult,
                            accum_out=aenc[:, d:d + 1])
                    rc = work.tile([128, 1], F32, name="rc", tag="rc")
                    nc.vector.reciprocal(rc, at[:, xh * 9 + 8:xh * 9 + 9])
                    tsum = work.tile([128, 8], F32, name="tsum", tag="tsum")
                    nc.vector.tensor_tensor(out=tsum, in0=at[:, xh * 9:xh * 9 + 8],
                                            in1=aenc, op=ALU.add)
                    if first:
                        nc.vector.tensor_scalar(aoT[xh][:, 8 * h:8 * h + 8], tsum,
                                                rc[:, 0:1], None, ALU.mult)
                    else:
                        nc.vector.tensor_scalar(aoT[xh][:, 8 * h:8 * h + 8], tsum,
                                                rc[:, 0:1], 0.0, ALU.mult, ALU.max)
            if first:
                for xt in range(nxt):
                    nc.sync.dma_start(out=oHT[n, xt * 128:(xt + 1) * 128, :],
                                      in_=aoT[xt])
            else:
                aop = ps_tp.tile([64, 128], F32, name="tpp", tag="tpp")
                nc.tensor.matmul(aop, aoT[0], ident_sb, is_transpose=True,
                                 start=True, stop=True)
                ao_sb = work.tile([64, 128], F32, name="ao_sb", tag="ao_sb")
                nc.scalar.activation(ao_sb, aop, AF.Copy)
                c3p = ps_mm.tile([64, 128], F32, name="mmp", tag="mmp")
                nc.tensor.matmul(c3p, w3t_sb, ao_sb, start=True, stop=True)
                t2 = work.tile([64, 128], F32, name="t2", tag="t2")
                nc.vector.scalar_tensor_tensor(t2, c3p, c3s_sb[:, 0:1],
                                               res_sb[:, n * 128:(n + 1) * 128],
                                               ALU.mult, ALU.add)
                on = work.tile([64, 128], F32, name="on", tag="on")
                nc.scalar.activation(on, t2, AF.Relu, bias=c3b_sb[:, 0:1], scale=1.0)
                nc.sync.dma_start(out=outn[:, n * 128:(n + 1) * 128], in_=on)
    return nc


def _run(build_fn, in_maps):
    nc = bacc.Bacc()
    build_fn(nc)
    nc.finalize()
    r = run_bass_kernel_spmd(nc, in_maps, list(range(NCORES)))
    return r.results


def kernel(x, conv1_w, bn1, kqv_w_h, kqv_bn_h, logits_bn_h, rel_enc_h,
           kqv_w_w, kqv_bn_w, logits_bn_w, rel_enc_w, conv3_w, bn3):
    x = np.asarray(x, np.float32)
    B, C, H, W = x.shape  # 1, 64, 256, 128
    wsh = W // NCORES     # 16
    hsh = H // NCORES     # 32

    s1c, b1c = _bn_sb(np.asarray(bn1))
    s3c, b3c = _bn_sb(np.asarray(bn3))
    fh = _axial_folds(np.asarray(kqv_w_h), np.asarray(kqv_bn_h),
                      np.asarray(logits_bn_h))
    fw = _axial_folds(np.asarray(kqv_w_w), np.asarray(kqv_bn_w),
                      np.asarray(logits_bn_w))
    (wkqvt_h, ks4_h, kb4_h, qs4_h, qb4_h, vs_h, vb_h, s2_h, s31_h) = fh
    (wkqvt_w, ks4_w, kb4_w, qs4_w, qb4_w, vs_w, vb_w, s2_w, s31_w) = fw
    qe_h, ke_h, ve_h = _enc_arrays(np.asarray(rel_enc_h, np.float32), 256)
    qe_w, ke_w, ve_w = _enc_arrays(np.asarray(rel_enc_w, np.float32), 128)
    ident = np.eye(128, dtype=np.float32)

    shared1 = dict(
        w1t=np.ascontiguousarray(np.asarray(conv1_w, np.float32).T),
        c1s=s1c.reshape(64, 1), c1b=b1c.reshape(64, 1),
        wkqvt=wkqvt_h, ks4=ks4_h, kb4=kb4_h, qs4=qs4_h, qb4=qb4_h,
        vsc=vs_h, vbc=vb_h,
        qenc=qe_h, kenc=ke_h, venc=ve_h, ident=ident)
    in_maps1 = []
    for c in range(NCORES):
        xs = np.ascontiguousarray(
            x[0, :, :, c * wsh:(c + 1) * wsh]).reshape(64, 256 * wsh)
        m = dict(shared1)
        m["xin"] = xs
        in_maps1.append(m)
    res1 = _run(lambda nc: _build_axial(nc, 256, wsh, True, s2_h, s31_h), in_maps1)

    # gather oHT [n(w), 256(H), 64(c)] per core -> oH [64, 256, 128]
    oH = np.empty((64, 256, 128), np.float32)
    for c in range(NCORES):
        t = res1[c]["oHT"]  # [wsh, 256, 64]
        oH[:, :, c * wsh:(c + 1) * wsh] = t.transpose(2, 1, 0)

    shared2 = dict(
        w3t=np.ascontiguousarray(np.asarray(conv3_w, np.float32).T),
        c3s=s3c.reshape(64, 1), c3b=b3c.reshape(64, 1),
        wkqvt=wkqvt_w, ks4=ks4_w, kb4=kb4_w, qs4=qs4_w, qb4=qb4_w,
        vsc=vs_w, vbc=vb_w,
        qenc=qe_w, kenc=ke_w, venc=ve_w, ident=ident)
    in_maps2 = []
    for c in range(NCORES):
        sl = slice(c * hsh, (c + 1) * hsh)
        m = dict(shared2)
        m["xin"] = np.ascontiguousarray(oH[:, sl, :]).reshape(64, hsh * 128)
        m["res"] = np.ascontiguousarray(x[0, :, sl, :]).reshape(64, hsh * 128)
        in_maps2.append(m)
    res2 = _run(lambda nc: _build_axial(nc, 128, hsh, False, s2_w, s31_w), in_maps2)

    out = np.empty((1, 64, 256, 128), np.float32)
    for c in range(NCORES):
        out[0, :, c * hsh:(c + 1) * hsh, :] = res2[c]["outn"].reshape(64, hsh, 128)
    return out


# revision 15
# speedup vs baseline: 2356.4450x; 2356.4450x over previous
"""Trainium2 Bass kernel for nn_AxialBottleneck (conv1x1+BN+relu -> axial-attn(H)
-> axial-attn(W) -> relu -> conv1x1+BN -> relu(+residual)).

Self-contained: accepts FULL inputs, shards across 8 NeuronCores internally
(axial-H sharded over W, axial-W sharded over H; two SPMD launches with a host
reshard between), returns the FULL output.
"""

import sys
from contextlib import ExitStack

import numpy as np

for _p in ("/opt/trn_rl_repo",):
    if _p not in sys.path:
        sys.path.insert(0, _p)

import concourse.bass as bass
import concourse.bacc as bacc
import concourse.mybir as mybir
import concourse.tile as tile
from concourse.bass_utils import run_bass_kernel_spmd

EPS = 1e-5
HEADS = 8
NCORES = 8
F32 = mybir.dt.float32
AF = mybir.ActivationFunctionType
ALU = mybir.AluOpType

# channel permutation: old kqv row 16h+i -> new row (k: 8i+h | q: 32+8(i-4)+h | v: 64+8(i-8)+h)
def _old_of_new():
    o = np.zeros(128, np.int64)
    for old in range(128):
        h, i = old // 16, old % 16
        if i < 4:
            new = 8 * i + h
        elif i < 8:
            new = 32 + 8 * (i - 4) + h
        else:
            new = 64 + 8 * (i - 8) + h
        o[new] = old
    return o


def _bn_sb(p):
    g, b, m, v = p.astype(np.float64)
    s = g / np.sqrt(v + EPS)
    return (s).astype(np.float32), (b - m * s).astype(np.float32)


def _enc_arrays(renc, L):
    # o[a, b] = b - a + L - 1
    o = np.arange(L)[None, :] - np.arange(L)[:, None] + L - 1
    qencS = renc[0:4][:, o]   # [4, x, y] = renc[d, y-x+L-1]
    kencST = renc[4:8][:, o]  # [4, y, x] = renc[4+d, x-y+L-1]
    vencS = renc[8:16][:, o]  # [8, x, y] = renc[8+d, y-x+L-1]
    return (np.ascontiguousarray(qencS, np.float32),
            np.ascontiguousarray(kencST, np.float32),
            np.ascontiguousarray(vencS, np.float32))


def _axial_folds(kqv_w, kqv_bn, logits_bn):
    """Permuted lhsT weight [64,128] (cols: k per-head 4h.., q 32+4h.., v 64+8h+d),
    per-head ACT scale/bias [4,8] for k and q, v scale/bias [64,1], s2[8], s31[8]."""
    sBN, bBN = _bn_sb(kqv_bn)          # [128]
    sL, _ = _bn_sb(logits_bn)          # [24]; bias part drops in softmax
    s1, s2, s3 = sL[0:8], sL[8:16], sL[16:24]
    old_k = np.array([16 * h + d for h in range(HEADS) for d in range(4)])
    old_q = np.array([16 * h + 4 + d for h in range(HEADS) for d in range(4)])
    old_v = np.array([16 * h + 8 + d for h in range(HEADS) for d in range(8)])
    order = np.concatenate([old_k, old_q, old_v])
    Wp = kqv_w[order]                  # [128, 64]
    ks4 = (sBN[old_k] * np.repeat(s1, 4)).reshape(8, 4).T.copy()   # [4, 8]
    kb4 = (bBN[old_k] * np.repeat(s1, 4)).reshape(8, 4).T.copy()
    qs4 = sBN[old_q].reshape(8, 4).T.copy()
    qb4 = bBN[old_q].reshape(8, 4).T.copy()
    vs = sBN[old_v].reshape(64, 1)
    vb = bBN[old_v].reshape(64, 1)
    return (np.ascontiguousarray(Wp.T, np.float32),
            ks4.astype(np.float32), kb4.astype(np.float32),
            qs4.astype(np.float32), qb4.astype(np.float32),
            vs.astype(np.float32), vb.astype(np.float32),
            s2.astype(np.float32), (s3 / s1).astype(np.float32))


def _build_axial(nc, L, n_per, first, s2, s31):
    """first=True: conv1 + axial-H (xin [64, 256*wsh] W-sliced; out oHT
    [n_per, 256, 64]). first=False: axial-W + relu + conv3 + bn3 + residual
    relu (xin/res [64, n_per*128] H-sliced; out outn [64, n_per*128])."""
    nyt = L // 128
    nxt = L // 128
    if first:
        wsh = 128 // NCORES
        xin = nc.dram_tensor("xin", [64, 256 * wsh], F32, kind="ExternalInput")
        w1t = nc.dram_tensor("w1t", [64, 64], F32, kind="ExternalInput")
        c1s = nc.dram_tensor("c1s", [64, 1], F32, kind="ExternalInput")
        c1b = nc.dram_tensor("c1b", [64, 1], F32, kind="ExternalInput")
        oHT = nc.dram_tensor("oHT", [n_per, 256, 64], F32, kind="ExternalOutput")
    else:
        xin = nc.dram_tensor("xin", [64, n_per * 128], F32, kind="ExternalInput")
        res = nc.dram_tensor("res", [64, n_per * 128], F32, kind="ExternalInput")
        w3t = nc.dram_tensor("w3t", [64, 64], F32, kind="ExternalInput")
        c3s = nc.dram_tensor("c3s", [64, 1], F32, kind="ExternalInput")
        c3b = nc.dram_tensor("c3b", [64, 1], F32, kind="ExternalInput")
        outn = nc.dram_tensor("outn", [64, n_per * 128], F32, kind="ExternalOutput")
    wkqvt = nc.dram_tensor("wkqvt", [64, 128], F32, kind="ExternalInput")
    ks4 = nc.dram_tensor("ks4", [4, 8], F32, kind="ExternalInput")
    kb4 = nc.dram_tensor("kb4", [4, 8], F32, kind="ExternalInput")
    qs4 = nc.dram_tensor("qs4", [4, 8], F32, kind="ExternalInput")
    qb4 = nc.dram_tensor("qb4", [4, 8], F32, kind="ExternalInput")
    vsc = nc.dram_tensor("vsc", [64, 1], F32, kind="ExternalInput")
    vbc = nc.dram_tensor("vbc", [64, 1], F32, kind="ExternalInput")
    qenc = nc.dram_tensor("qenc", [4, L, L], F32, kind="ExternalInput")
    kenc = nc.dram_tensor("kenc", [4, L, L], F32, kind="ExternalInput")
    venc = nc.dram_tensor("venc", [8, L, L], F32, kind="ExternalInput")
    ident = nc.dram_tensor("ident", [128, 128], F32, kind="ExternalInput")

    with ExitStack() as ctx:
        tc = ctx.enter_context(tile.TileContext(nc))
        singles = ctx.enter_context(tc.tile_pool(name="singles", bufs=1))
        work = ctx.enter_context(tc.tile_pool(name="work", bufs=3))
        wide = ctx.enter_context(tc.tile_pool(name="wide", bufs=2))
        ps_mm = ctx.enter_context(tc.tile_pool(name="ps_mm", bufs=2, space="PSUM"))
        ps_lt = ctx.enter_context(tc.tile_pool(name="ps_lt", bufs=2, space="PSUM"))
        ps_el = ctx.enter_context(tc.tile_pool(name="ps_el", bufs=2, space="PSUM"))
        ps_at = ctx.enter_context(tc.tile_pool(name="ps_at", bufs=1, space="PSUM"))
        ps_tp = ctx.enter_context(tc.tile_pool(name="ps_tp", bufs=1, space="PSUM"))

        def load(name, dram, shape):
            t = singles.tile(shape, F32, name=name, tag=name)
            nc.sync.dma_start(out=t, in_=dram[:, :] if len(shape) == 2 else dram)
            return t

        ident_sb = load("ident", ident, [128, 128])
        wkqvt_d = load("wkqvt", wkqvt, [64, 128])
        wkqvt_sb = singles.tile([64, 128], F32, name="wkqvt_c", tag="wkqvt_c")
        nc.vector.tensor_scalar(wkqvt_sb, wkqvt_d, 1.0, None, ALU.mult)
        ks4_sb = load("ks4", ks4, [4, 8])
        kb4_sb = load("kb4", kb4, [4, 8])
        qs4_sb = load("qs4", qs4, [4, 8])
        qb4_sb = load("qb4", qb4, [4, 8])
        vsc_sb = load("vsc", vsc, [64, 1])
        vbc_sb = load("vbc", vbc, [64, 1])
        qenc_sb = {}
        kenc_sb = {}
        venc_sb = {}
        for d in range(4):
            for xt in range(nxt):
                t = singles.tile([128, L], F32, name=f"qe{d}_{xt}", tag=f"qe{d}_{xt}")
                nc.sync.dma_start(out=t, in_=qenc[d, xt * 128:(xt + 1) * 128, :])
                qenc_sb[(d, xt)] = t
            for yt in range(nyt):
                t = singles.tile([128, L], F32, name=f"ke{d}_{yt}", tag=f"ke{d}_{yt}")
                nc.sync.dma_start(out=t, in_=kenc[d, yt * 128:(yt + 1) * 128, :])
                kenc_sb[(d, yt)] = t
        for d in range(8):
            for xt in range(nxt):
                t = singles.tile([128, L], F32, name=f"ve{d}_{xt}", tag=f"ve{d}_{xt}")
                nc.sync.dma_start(out=t, in_=venc[d, xt * 128:(xt + 1) * 128, :])
                venc_sb[(d, xt)] = t

        if first:
            w1t_d = load("w1t", w1t, [64, 64])
            w1t_sb = singles.tile([64, 64], F32, name="w1t_c", tag="w1t_c")
            nc.vector.tensor_scalar(w1t_sb, w1t_d, 1.0, None, ALU.mult)
            c1s_sb = load("c1s", c1s, [64, 1])
            c1b_sb = load("c1b", c1b, [64, 1])
            wsh = 128 // NCORES
            npx = 256 * wsh
            xin_sb = load("xin", xin, [64, npx])
            conv_sb = singles.tile([64, npx], F32, name="conv", tag="conv")
            for ck in range(npx // 512):
                cp = ps_mm.tile([64, 512], F32, name="mmp", tag="mmp")
                nc.tensor.matmul(cp, w1t_sb, xin_sb[:, ck * 512:(ck + 1) * 512],
                                 start=True, stop=True)
                nc.scalar.activation(conv_sb[:, ck * 512:(ck + 1) * 512], cp,
                                     AF.Relu, bias=c1b_sb[:, 0:1], scale=c1s_sb[:, 0:1])
        else:
            w3t_d = load("w3t", w3t, [64, 64])
            w3t_sb = singles.tile([64, 64], F32, name="w3t_c", tag="w3t_c")
            nc.vector.tensor_scalar(w3t_sb, w3t_d, 1.0, None, ALU.mult)
            c3s_sb = load("c3s", c3s, [64, 1])
            c3b_sb = load("c3b", c3b, [64, 1])
            xin_sb = load("xin", xin, [64, n_per * 128])
            res_sb = load("res", res, [64, n_per * 128])

        for n in range(n_per):
            if first:
                wsh = 128 // NCORES
                xf = conv_sb[:, n:256 * wsh:wsh]
            else:
                xf = xin_sb[:, n * 128:(n + 1) * 128]
            # per-head k, q tiles [4, L]; v tile [64, L]
            kh, qh = [], []
            for h in range(HEADS):
                kp = ps_mm.tile([4, L], F32, name="mmp", tag="mmp")
                nc.tensor.matmul(kp, wkqvt_sb[:, 4 * h:4 * h + 4], xf,
                                 start=True, stop=True)
                kt = work.tile([4, L], F32, name="kh", tag="kh", bufs=10)
                nc.scalar.activation(kt, kp, AF.Identity,
                                     bias=kb4_sb[:, h:h + 1], scale=ks4_sb[:, h:h + 1])
                kh.append(kt)
                qp = ps_mm.tile([4, L], F32, name="mmp", tag="mmp")
                nc.tensor.matmul(qp, wkqvt_sb[:, 32 + 4 * h:32 + 4 * h + 4], xf,
                                 start=True, stop=True)
                qt = work.tile([4, L], F32, name="qh", tag="qh", bufs=10)
                nc.scalar.activation(qt, qp, AF.Identity,
                                     bias=qb4_sb[:, h:h + 1], scale=qs4_sb[:, h:h + 1])
                qh.append(qt)
            vp = ps_mm.tile([64, L], F32, name="mmp", tag="mmp")
            nc.tensor.matmul(vp, wkqvt_sb[:, 64:128], xf, start=True, stop=True)
            vt = wide.tile([64, L], F32, name="vt", tag="vt", bufs=2)
            nc.scalar.activation(vt, vp, AF.Identity,
                                 bias=vbc_sb[:, 0:1], scale=vsc_sb[:, 0:1])
            vT = []
            for yt in range(nyt):
                tp = ps_tp.tile([128, 64], F32, name="tpp", tag="tpp")
                nc.tensor.matmul(tp, vt[:, yt * 128:(yt + 1) * 128],
                                 ident_sb[0:64, 0:64], is_transpose=True,
                                 start=True, stop=True)
                ts = work.tile([128, 64], F32, name="vT", tag="vT", bufs=4)
                nc.scalar.activation(ts, tp, AF.Copy)
                vT.append(ts)
            aoT = []
            for xt in range(nxt):
                aoT.append(work.tile([128, 64], F32, name=f"aoT{xt}", tag=f"aoT{xt}"))
            for h in range(HEADS):
                ksc, qsc = [], []
                for yt in range(nyt):
                    tp = ps_tp.tile([128, 4], F32, name="tpp", tag="tpp")
                    nc.tensor.matmul(tp, kh[h][:, yt * 128:(yt + 1) * 128],
                                     ident_sb[0:4, 0:4], is_transpose=True,
                                     start=True, stop=True)
                    t = work.tile([128, 4], F32, name="ksc", tag="ksc", bufs=4)
                    nc.scalar.activation(t, tp, AF.Copy, scale=float(s31[h]))
                    ksc.append(t)
                for xt in range(nxt):
                    tp = ps_tp.tile([128, 4], F32, name="tpp", tag="tpp")
                    nc.tensor.matmul(tp, qh[h][:, xt * 128:(xt + 1) * 128],
                                     ident_sb[0:4, 0:4], is_transpose=True,
                                     start=True, stop=True)
                    t = work.tile([128, 4], F32, name="qsc", tag="qsc", bufs=4)
                    nc.scalar.activation(t, tp, AF.Copy, scale=float(s2[h]))
                    qsc.append(t)
                # qr natural [x, y]
                qr = []
                for xt in range(nxt):
                    t = wide.tile([128, L], F32, name="qr", tag="qr")
                    nc.vector.tensor_scalar(t, qenc_sb[(0, xt)], qsc[xt][:, 0:1],
                                            None, ALU.mult)
                    for d in range(1, 4):
                        nc.vector.scalar_tensor_tensor(
                            t, qenc_sb[(d, xt)], qsc[xt][:, d:d + 1], t,
                            ALU.mult, ALU.add)
                    qr.append(t)
                # logits^T [y, x] in PSUM: qk + qr^T + kr
                LT = []
                for yt in range(nyt):
                    lt = ps_lt.tile([128, L], F32, name="lt", tag="lt")
                    nc.tensor.matmul(lt, kh[h][:, yt * 128:(yt + 1) * 128],
                                     qh[h], start=True, stop=False)
                    for xt in range(nxt):
                        nc.tensor.matmul(lt[:, xt * 128:(xt + 1) * 128],
                                         qr[xt][:, yt * 128:(yt + 1) * 128],
                                         ident_sb, is_transpose=True,
                                         start=False, stop=(xt == nxt - 1))
                    for d in range(4):
                        nc.vector.scalar_tensor_tensor(
                            lt, kenc_sb[(d, yt)], ksc[yt][:, d:d + 1], lt,
                            ALU.mult, ALU.add)
                    LT.append(lt)
                expLT = []
                for yt in range(nyt):
                    e = wide.tile([128, L], F32, name="explt", tag="explt", bufs=4)
                    nc.scalar.activation(e, LT[yt], AF.Exp)
                    expLT.append(e)
                arhs = []
                for yt in range(nyt):
                    r = work.tile([128, 9], F32, name="arhs", tag="arhs", bufs=4)
                    nc.vector.tensor_scalar(r[:, 0:8], vT[yt][:, 8 * h:8 * h + 8],
                                            1.0, None, ALU.mult)
                    nc.vector.memset(r[:, 8:9], 1.0)
                    arhs.append(r)
                at = ps_at.tile([128, 9 * nxt], F32, name="at", tag="at")
                for xh in range(nxt):
                    for yt in range(nyt):
                        nc.tensor.matmul(at[:, xh * 9:xh * 9 + 9],
                                         expLT[yt][:, xh * 128:(xh + 1) * 128],
                                         arhs[yt], start=(yt == 0),
                                         stop=(yt == nyt - 1))
                for xh in range(nxt):
                    ep = ps_el.tile([128, L], F32, name="expl", tag="expl")
                    for yt in range(nyt):
                        nc.tensor.matmul(ep[:, yt * 128:(yt + 1) * 128],
                                         expLT[yt][:, xh * 128:(xh + 1) * 128],
                                         ident_sb, is_transpose=True,
                                         start=True, stop=True)
                    aenc = work.tile([128, 8], F32, name="aenc", tag="aenc")
                    scratch = wide.tile([128, L], F32, name="scratch", tag="scratch")
                    for d in range(8):
                        nc.vector.scalar_tensor_tensor(
                            scratch, venc_sb[(d, xh)], 1.0, ep,
                            ALU.mult, ALU.mult,
                            accum_out=aenc[:, d:d + 1])
                    rc = work.tile([128, 1], F32, name="rc", tag="rc")
                    nc.vector.reciprocal(rc, at[:, xh * 9 + 8:xh * 9 + 9])
                    tsum = work.tile([128, 8], F32, name="tsum", tag="tsum")
                    nc.vector.tensor_tensor(out=tsum, in0=at[:, xh * 9:xh * 9 + 8],
                                            in1=aenc, op=ALU.add)
                    if first:
                        nc.vector.tensor_scalar(aoT[xh][:, 8 * h:8 * h + 8], tsum,
                                                rc[:, 0:1], None, ALU.mult)
                    else:
                        nc.vector.tensor_scalar(aoT[xh][:, 8 * h:8 * h + 8], tsum,
                                                rc[:, 0:1], 0.0, ALU.mult, ALU.max)
            if first:
                for xt in range(nxt):
                    nc.sync.dma_start(out=oHT[n, xt * 128:(xt + 1) * 128, :],
                                      in_=aoT[xt])
            else:
                aop = ps_tp.tile([64, 128], F32, name="tpp", tag="tpp")
                nc.tensor.matmul(aop, aoT[0], ident_sb, is_transpose=True,
                                 start=True, stop=True)
                ao_sb = work.tile([64, 128], F32, name="ao_sb", tag="ao_sb")
                nc.scalar.activation(ao_sb, aop, AF.Copy)
                c3p = ps_mm.tile([64, 128], F32, name="mmp", tag="mmp")
                nc.tensor.matmul(c3p, w3t_sb, ao_sb, start=True, stop=True)
                t2 = work.tile([64, 128], F32, name="t2", tag="t2")
                nc.vector.scalar_tensor_tensor(t2, c3p, c3s_sb[:, 0:1],
                                               res_sb[:, n * 128:(n + 1) * 128],
                                               ALU.mult, ALU.add)
                on = work.tile([64, 128], F32, name="on", tag="on")
                nc.scalar.activation(on, t2, AF.Relu, bias=c3b_sb[:, 0:1], scale=1.0)
                nc.sync.dma_start(out=outn[:, n * 128:(n + 1) * 128], in_=on)
    return nc


def _run(build_fn, in_maps):
    nc = bacc.Bacc()
    build_fn(nc)
    nc.finalize()
    import os
    tr = bool(int(os.environ.get("BASS_KERNEL_TRACE", "0")))
    r = run_bass_kernel_spmd(nc, in_maps, list(range(NCORES)), trace=tr)
    if tr and r.exec_time_ns is not None:
        print("launch exec_time_ns:", r.exec_time_ns)
    return r.results


def kernel(x, conv1_w, bn1, kqv_w_h, kqv_bn_h, logits_bn_h, rel_enc_h,
           kqv_w_w, kqv_bn_w, logits_bn_w, rel_enc_w, conv3_w, bn3):
    x = np.asarray(x, np.float32)
    B, C, H, W = x.shape  # 1, 64, 256, 128
    wsh = W // NCORES     # 16
    hsh = H // NCORES     # 32

    s1c, b1c = _bn_sb(np.asarray(bn1))
    s3c, b3c = _bn_sb(np.asarray(bn3))
    fh = _axial_folds(np.asarray(kqv_w_h), np.asarray(kqv_bn_h),
                      np.asarray(logits_bn_h))
    fw = _axial_folds(np.asarray(kqv_w_w), np.asarray(kqv_bn_w),
                      np.asarray(logits_bn_w))
    (wkqvt_h, ks4_h, kb4_h, qs4_h, qb4_h, vs_h, vb_h, s2_h, s31_h) = fh
    (wkqvt_w, ks4_w, kb4_w, qs4_w, qb4_w, vs_w, vb_w, s2_w, s31_w) = fw
    qe_h, ke_h, ve_h = _enc_arrays(np.asarray(rel_enc_h, np.float32), 256)
    qe_w, ke_w, ve_w = _enc_arrays(np.asarray(rel_enc_w, np.float32), 128)
    ident = np.eye(128, dtype=np.float32)

    shared1 = dict(
        w1t=np.ascontiguousarray(np.asarray(conv1_w, np.float32).T),
        c1s=s1c.reshape(64, 1), c1b=b1c.reshape(64, 1),
        wkqvt=wkqvt_h, ks4=ks4_h, kb4=kb4_h, qs4=qs4_h, qb4=qb4_h,
        vsc=vs_h, vbc=vb_h,
        qenc=qe_h, kenc=ke_h, venc=ve_h, ident=ident)
    in_maps1 = []
    for c in range(NCORES):
        xs = np.ascontiguousarray(
            x[0, :, :, c * wsh:(c + 1) * wsh]).reshape(64, 256 * wsh)
        m = dict(shared1)
        m["xin"] = xs
        in_maps1.append(m)
    res1 = _run(lambda nc: _build_axial(nc, 256, wsh, True, s2_h, s31_h), in_maps1)

    # gather oHT [n(w), 256(H), 64(c)] per core -> oH [64, 256, 128]
    oH = np.empty((64, 256, 128), np.float32)
    for c in range(NCORES):
        t = res1[c]["oHT"]  # [wsh, 256, 64]
        oH[:, :, c * wsh:(c + 1) * wsh] = t.transpose(2, 1, 0)

    shared2 = dict(
        w3t=np.ascontiguousarray(np.asarray(conv3_w, np.float32).T),
        c3s=s3c.reshape(64, 1), c3b=b3c.reshape(64, 1),
        wkqvt=wkqvt_w, ks4=ks4_w, kb4=kb4_w, qs4=qs4_w, qb4=qb4_w,
        vsc=vs_w, vbc=vb_w,
        qenc=qe_w, kenc=ke_w, venc=ve_w, ident=ident)
    in_maps2 = []
    for c in range(NCORES):
        sl = slice(c * hsh, (c + 1) * hsh)
        m = dict(shared2)
        m["xin"] = np.ascontiguousarray(oH[:, sl, :]).reshape(64, hsh * 128)
        m["res"] = np.ascontiguousarray(x[0, :, sl, :]).reshape(64, hsh * 128)
        in_maps2.append(m)
    res2 = _run(lambda nc: _build_axial(nc, 128, hsh, False, s2_w, s31_w), in_maps2)

    out = np.empty((1, 64, 256, 128), np.float32)
    for c in range(NCORES):
        out[0, :, c * hsh:(c + 1) * hsh, :] = res2[c]["outn"].reshape(64, hsh, 128)
    return out


# revision 16
# speedup vs baseline: 2581.4295x; 1.0955x over previous
"""Trainium2 Bass kernel for nn_AxialBottleneck (conv1x1+BN+relu -> axial-attn(H)
-> axial-attn(W) -> relu -> conv1x1+BN -> relu(+residual)).

Self-contained: accepts FULL inputs, shards across 8 NeuronCores internally
(axial-H sharded over W, axial-W sharded over H; two SPMD launches with a host
reshard between), returns the FULL output.
"""

import sys
from contextlib import ExitStack

import numpy as np

for _p in ("/opt/trn_rl_repo",):
    if _p not in sys.path:
        sys.path.insert(0, _p)

import concourse.bass as bass
import concourse.bacc as bacc
import concourse.mybir as mybir
import concourse.tile as tile
from concourse.bass_utils import run_bass_kernel_spmd

def _bf16(a):
    import ml_dtypes
    return a.astype(ml_dtypes.bfloat16)


EPS = 1e-5
HEADS = 8
NCORES = 8
F32 = mybir.dt.float32
BF16 = mybir.dt.bfloat16
AF = mybir.ActivationFunctionType
ALU = mybir.AluOpType

# channel permutation: old kqv row 16h+i -> new row (k: 8i+h | q: 32+8(i-4)+h | v: 64+8(i-8)+h)
def _old_of_new():
    o = np.zeros(128, np.int64)
    for old in range(128):
        h, i = old // 16, old % 16
        if i < 4:
            new = 8 * i + h
        elif i < 8:
            new = 32 + 8 * (i - 4) + h
        else:
            new = 64 + 8 * (i - 8) + h
        o[new] = old
    return o


def _bn_sb(p):
    g, b, m, v = p.astype(np.float64)
    s = g / np.sqrt(v + EPS)
    return (s).astype(np.float32), (b - m * s).astype(np.float32)


def _enc_arrays(renc, L):
    # o[a, b] = b - a + L - 1
    o = np.arange(L)[None, :] - np.arange(L)[:, None] + L - 1
    qencS = renc[0:4][:, o]   # [4, x, y] = renc[d, y-x+L-1]
    kencST = renc[4:8][:, o]  # [4, y, x] = renc[4+d, x-y+L-1]
    vencS = renc[8:16][:, o]  # [8, x, y] = renc[8+d, y-x+L-1]
    return (np.ascontiguousarray(qencS, np.float32),
            np.ascontiguousarray(kencST, np.float32),
            np.ascontiguousarray(vencS, np.float32))


def _axial_folds(kqv_w, kqv_bn, logits_bn):
    """Permuted lhsT weight [64,128] (cols: k per-head 4h.., q 32+4h.., v 64+8h+d),
    per-head ACT scale/bias [4,8] for k and q, v scale/bias [64,1], s2[8], s31[8]."""
    sBN, bBN = _bn_sb(kqv_bn)          # [128]
    sL, _ = _bn_sb(logits_bn)          # [24]; bias part drops in softmax
    s1, s2, s3 = sL[0:8], sL[8:16], sL[16:24]
    old_k = np.array([16 * h + d for h in range(HEADS) for d in range(4)])
    old_q = np.array([16 * h + 4 + d for h in range(HEADS) for d in range(4)])
    old_v = np.array([16 * h + 8 + d for h in range(HEADS) for d in range(8)])
    order = np.concatenate([old_k, old_q, old_v])
    Wp = kqv_w[order]                  # [128, 64]
    ks4 = (sBN[old_k] * np.repeat(s1, 4)).reshape(8, 4).T.copy()   # [4, 8]
    kb4 = (bBN[old_k] * np.repeat(s1, 4)).reshape(8, 4).T.copy()
    qs4 = sBN[old_q].reshape(8, 4).T.copy()
    qb4 = bBN[old_q].reshape(8, 4).T.copy()
    vs = sBN[old_v].reshape(64, 1)
    vb = bBN[old_v].reshape(64, 1)
    return (np.ascontiguousarray(Wp.T, np.float32),
            ks4.astype(np.float32), kb4.astype(np.float32),
            qs4.astype(np.float32), qb4.astype(np.float32),
            vs.astype(np.float32), vb.astype(np.float32),
            s2.astype(np.float32), (s3 / s1).astype(np.float32))


def _build_axial(nc, L, n_per, first, s2, s31):
    """first=True: conv1 + axial-H (xin [64, 256*wsh] W-sliced; out oHT
    [n_per, 256, 64]). first=False: axial-W + relu + conv3 + bn3 + residual
    relu (xin/res [64, n_per*128] H-sliced; out outn [64, n_per*128])."""
    nyt = L // 128
    nxt = L // 128
    if first:
        wsh = 128 // NCORES
        xin = nc.dram_tensor("xin", [64, 256 * wsh], F32, kind="ExternalInput")
        w1t = nc.dram_tensor("w1t", [64, 64], F32, kind="ExternalInput")
        c1s = nc.dram_tensor("c1s", [64, 1], F32, kind="ExternalInput")
        c1b = nc.dram_tensor("c1b", [64, 1], F32, kind="ExternalInput")
        oHT = nc.dram_tensor("oHT", [n_per, 256, 64], F32, kind="ExternalOutput")
    else:
        xin = nc.dram_tensor("xin", [64, n_per * 128], F32, kind="ExternalInput")
        res = nc.dram_tensor("res", [64, n_per * 128], F32, kind="ExternalInput")
        w3t = nc.dram_tensor("w3t", [64, 64], F32, kind="ExternalInput")
        c3s = nc.dram_tensor("c3s", [64, 1], F32, kind="ExternalInput")
        c3b = nc.dram_tensor("c3b", [64, 1], F32, kind="ExternalInput")
        outn = nc.dram_tensor("outn", [64, n_per * 128], F32, kind="ExternalOutput")
    wkqvt = nc.dram_tensor("wkqvt", [64, 128], F32, kind="ExternalInput")
    ks4 = nc.dram_tensor("ks4", [4, 8], F32, kind="ExternalInput")
    kb4 = nc.dram_tensor("kb4", [4, 8], F32, kind="ExternalInput")
    qs4 = nc.dram_tensor("qs4", [4, 8], F32, kind="ExternalInput")
    qb4 = nc.dram_tensor("qb4", [4, 8], F32, kind="ExternalInput")
    vsc = nc.dram_tensor("vsc", [64, 1], F32, kind="ExternalInput")
    vbc = nc.dram_tensor("vbc", [64, 1], F32, kind="ExternalInput")
    qenc = nc.dram_tensor("qenc", [4, L, L], F32, kind="ExternalInput")
    kenc = nc.dram_tensor("kenc", [4, L, L], F32, kind="ExternalInput")
    venc = nc.dram_tensor("venc", [8, L, L], BF16, kind="ExternalInput")
    ident = nc.dram_tensor("ident", [128, 128], F32, kind="ExternalInput")

    with ExitStack() as ctx:
        tc = ctx.enter_context(tile.TileContext(nc))
        singles = ctx.enter_context(tc.tile_pool(name="singles", bufs=1))
        work = ctx.enter_context(tc.tile_pool(name="work", bufs=3))
        wide = ctx.enter_context(tc.tile_pool(name="wide", bufs=2))
        ps_mm = ctx.enter_context(tc.tile_pool(name="ps_mm", bufs=2, space="PSUM"))
        ps_lt = ctx.enter_context(tc.tile_pool(name="ps_lt", bufs=2, space="PSUM"))
        ps_el = ctx.enter_context(tc.tile_pool(name="ps_el", bufs=2, space="PSUM"))
        ps_at = ctx.enter_context(tc.tile_pool(name="ps_at", bufs=1, space="PSUM"))
        ps_tp = ctx.enter_context(tc.tile_pool(name="ps_tp", bufs=1, space="PSUM"))

        def load(name, dram, shape):
            t = singles.tile(shape, F32, name=name, tag=name)
            nc.sync.dma_start(out=t, in_=dram[:, :] if len(shape) == 2 else dram)
            return t

        ident_sb = load("ident", ident, [128, 128])
        wkqvt_d = load("wkqvt", wkqvt, [64, 128])
        wkqvt_sb = singles.tile([64, 128], F32, name="wkqvt_c", tag="wkqvt_c")
        nc.vector.tensor_scalar(wkqvt_sb, wkqvt_d, 1.0, None, ALU.mult)
        ks4_sb = load("ks4", ks4, [4, 8])
        kb4_sb = load("kb4", kb4, [4, 8])
        qs4_sb = load("qs4", qs4, [4, 8])
        qb4_sb = load("qb4", qb4, [4, 8])
        vsc_sb = load("vsc", vsc, [64, 1])
        vbc_sb = load("vbc", vbc, [64, 1])
        qenc_sb = {}
        kenc_sb = {}
        venc_sb = {}
        for d in range(4):
            for xt in range(nxt):
                t = singles.tile([128, L], F32, name=f"qe{d}_{xt}", tag=f"qe{d}_{xt}")
                nc.sync.dma_start(out=t, in_=qenc[d, xt * 128:(xt + 1) * 128, :])
                qenc_sb[(d, xt)] = t
            for yt in range(nyt):
                t = singles.tile([128, L], F32, name=f"ke{d}_{yt}", tag=f"ke{d}_{yt}")
                nc.sync.dma_start(out=t, in_=kenc[d, yt * 128:(yt + 1) * 128, :])
                kenc_sb[(d, yt)] = t
        for d in range(8):
            for xt in range(nxt):
                t = singles.tile([128, L], BF16, name=f"ve{d}_{xt}", tag=f"ve{d}_{xt}")
                nc.sync.dma_start(out=t, in_=venc[d, xt * 128:(xt + 1) * 128, :])
                venc_sb[(d, xt)] = t

        if first:
            w1t_d = load("w1t", w1t, [64, 64])
            w1t_sb = singles.tile([64, 64], F32, name="w1t_c", tag="w1t_c")
            nc.vector.tensor_scalar(w1t_sb, w1t_d, 1.0, None, ALU.mult)
            c1s_sb = load("c1s", c1s, [64, 1])
            c1b_sb = load("c1b", c1b, [64, 1])
            wsh = 128 // NCORES
            npx = 256 * wsh
            xin_sb = load("xin", xin, [64, npx])
            conv_sb = singles.tile([64, npx], F32, name="conv", tag="conv")
            for ck in range(npx // 512):
                cp = ps_mm.tile([64, 512], F32, name="mmp", tag="mmp")
                nc.tensor.matmul(cp, w1t_sb, xin_sb[:, ck * 512:(ck + 1) * 512],
                                 start=True, stop=True)
                nc.scalar.activation(conv_sb[:, ck * 512:(ck + 1) * 512], cp,
                                     AF.Relu, bias=c1b_sb[:, 0:1], scale=c1s_sb[:, 0:1])
        else:
            w3t_d = load("w3t", w3t, [64, 64])
            w3t_sb = singles.tile([64, 64], F32, name="w3t_c", tag="w3t_c")
            nc.vector.tensor_scalar(w3t_sb, w3t_d, 1.0, None, ALU.mult)
            c3s_sb = load("c3s", c3s, [64, 1])
            c3b_sb = load("c3b", c3b, [64, 1])
            xin_sb = load("xin", xin, [64, n_per * 128])
            res_sb = load("res", res, [64, n_per * 128])

        for n in range(n_per):
            if first:
                wsh = 128 // NCORES
                xf = conv_sb[:, n:256 * wsh:wsh]
            else:
                xf = xin_sb[:, n * 128:(n + 1) * 128]
            # per-head k, q tiles [4, L]; v tile [64, L]
            kh, qh = [], []
            for h in range(HEADS):
                kp = ps_mm.tile([4, L], F32, name="mmp", tag="mmp")
                nc.tensor.matmul(kp, wkqvt_sb[:, 4 * h:4 * h + 4], xf,
                                 start=True, stop=True)
                kt = work.tile([4, L], F32, name="kh", tag="kh", bufs=10)
                nc.scalar.activation(kt, kp, AF.Identity,
                                     bias=kb4_sb[:, h:h + 1], scale=ks4_sb[:, h:h + 1])
                kh.append(kt)
                qp = ps_mm.tile([4, L], F32, name="mmp", tag="mmp")
                nc.tensor.matmul(qp, wkqvt_sb[:, 32 + 4 * h:32 + 4 * h + 4], xf,
                                 start=True, stop=True)
                qt = work.tile([4, L], F32, name="qh", tag="qh", bufs=10)
                nc.scalar.activation(qt, qp, AF.Identity,
                                     bias=qb4_sb[:, h:h + 1], scale=qs4_sb[:, h:h + 1])
                qh.append(qt)
            vp = ps_mm.tile([64, L], F32, name="mmp", tag="mmp")
            nc.tensor.matmul(vp, wkqvt_sb[:, 64:128], xf, start=True, stop=True)
            vt = wide.tile([64, L], F32, name="vt", tag="vt", bufs=2)
            nc.scalar.activation(vt, vp, AF.Identity,
                                 bias=vbc_sb[:, 0:1], scale=vsc_sb[:, 0:1])
            vT = []
            for yt in range(nyt):
                tp = ps_tp.tile([128, 64], F32, name="tpp", tag="tpp")
                nc.tensor.matmul(tp, vt[:, yt * 128:(yt + 1) * 128],
                                 ident_sb[0:64, 0:64], is_transpose=True,
                                 start=True, stop=True)
                ts = work.tile([128, 64], F32, name="vT", tag="vT", bufs=4)
                nc.scalar.activation(ts, tp, AF.Copy)
                vT.append(ts)
            aoT = []
            for xt in range(nxt):
                aoT.append(work.tile([128, 64], F32, name=f"aoT{xt}", tag=f"aoT{xt}"))
            for h in range(HEADS):
                ksc, qsc = [], []
                for yt in range(nyt):
                    tp = ps_tp.tile([128, 4], F32, name="tpp", tag="tpp")
                    nc.tensor.matmul(tp, kh[h][:, yt * 128:(yt + 1) * 128],
                                     ident_sb[0:4, 0:4], is_transpose=True,
                                     start=True, stop=True)
                    t = work.tile([128, 4], F32, name="ksc", tag="ksc", bufs=4)
                    nc.scalar.activation(t, tp, AF.Copy, scale=float(s31[h]))
                    ksc.append(t)
                for xt in range(nxt):
                    tp = ps_tp.tile([128, 4], F32, name="tpp", tag="tpp")
                    nc.tensor.matmul(tp, qh[h][:, xt * 128:(xt + 1) * 128],
                                     ident_sb[0:4, 0:4], is_transpose=True,
                                     start=True, stop=True)
                    t = work.tile([128, 4], F32, name="qsc", tag="qsc", bufs=4)
                    nc.scalar.activation(t, tp, AF.Copy, scale=float(s2[h]))
                    qsc.append(t)
                # qr natural [x, y]
                qr = []
                for xt in range(nxt):
                    t = wide.tile([128, L], F32, name="qr", tag="qr")
                    nc.vector.tensor_scalar(t, qenc_sb[(0, xt)], qsc[xt][:, 0:1],
                                            None, ALU.mult)
                    for d in range(1, 4):
                        nc.vector.scalar_tensor_tensor(
                            t, qenc_sb[(d, xt)], qsc[xt][:, d:d + 1], t,
                            ALU.mult, ALU.add)
                    qr.append(t)
                # logits^T [y, x] in PSUM: qk + qr^T + kr
                LT = []
                for yt in range(nyt):
                    lt = ps_lt.tile([128, L], F32, name="lt", tag="lt")
                    nc.tensor.matmul(lt, kh[h][:, yt * 128:(yt + 1) * 128],
                                     qh[h], start=True, stop=False)
                    for xt in range(nxt):
                        nc.tensor.matmul(lt[:, xt * 128:(xt + 1) * 128],
                                         qr[xt][:, yt * 128:(yt + 1) * 128],
                                         ident_sb, is_transpose=True,
                                         start=False, stop=(xt == nxt - 1))
                    for d in range(4):
                        nc.vector.scalar_tensor_tensor(
                            lt, kenc_sb[(d, yt)], ksc[yt][:, d:d + 1], lt,
                            ALU.mult, ALU.add)
                    LT.append(lt)
                expLT = []
                for yt in range(nyt):
                    e = wide.tile([128, L], F32, name="explt", tag="explt", bufs=4)
                    nc.scalar.activation(e, LT[yt], AF.Exp)
                    expLT.append(e)
                arhs = []
                for yt in range(nyt):
                    r = work.tile([128, 9], F32, name="arhs", tag="arhs", bufs=4)
                    nc.vector.tensor_scalar(r[:, 0:8], vT[yt][:, 8 * h:8 * h + 8],
                                            1.0, None, ALU.mult)
                    nc.vector.memset(r[:, 8:9], 1.0)
                    arhs.append(r)
                at = ps_at.tile([128, 9 * nxt], F32, name="at", tag="at")
                for xh in range(nxt):
                    for yt in range(nyt):
                        nc.tensor.matmul(at[:, xh * 9:xh * 9 + 9],
                                         expLT[yt][:, xh * 128:(xh + 1) * 128],
                                         arhs[yt], start=(yt == 0),
                                         stop=(yt == nyt - 1))
                for xh in range(nxt):
                    ep = ps_el.tile([128, L], F32, name="expl", tag="expl")
                    for yt in range(nyt):
                        nc.tensor.matmul(ep[:, yt * 128:(yt + 1) * 128],
                                         expLT[yt][:, xh * 128:(xh + 1) * 128],
                                         ident_sb, is_transpose=True,
                                         start=True, stop=True)
                    epb = wide.tile([128, L], BF16, name="epb", tag="epb")
                    nc.scalar.activation(epb, ep, AF.Copy)
                    aenc = work.tile([128, 8], F32, name="aenc", tag="aenc")
                    scratch = wide.tile([128, L], BF16, name="scratch", tag="scratch")
                    for d in range(8):
                        nc.vector.scalar_tensor_tensor(
                            scratch, venc_sb[(d, xh)], 1.0, epb,
                            ALU.mult, ALU.mult,
                            accum_out=aenc[:, d:d + 1])
                    rc = work.tile([128, 1], F32, name="rc", tag="rc")
                    nc.vector.reciprocal(rc, at[:, xh * 9 + 8:xh * 9 + 9])
                    tsum = work.tile([128, 8], F32, name="tsum", tag="tsum")
                    nc.vector.tensor_tensor(out=tsum, in0=at[:, xh * 9:xh * 9 + 8],
                                            in1=aenc, op=ALU.add)
                    if first:
                        nc.vector.tensor_scalar(aoT[xh][:, 8 * h:8 * h + 8], tsum,
                                                rc[:, 0:1], None, ALU.mult)
                    else:
                        nc.vector.tensor_scalar(aoT[xh][:, 8 * h:8 * h + 8], tsum,
                                                rc[:, 0:1], 0.0, ALU.mult, ALU.max)
            if first:
                for xt in range(nxt):
                    nc.sync.dma_start(out=oHT[n, xt * 128:(xt + 1) * 128, :],
                                      in_=aoT[xt])
            else:
                aop = ps_tp.tile([64, 128], F32, name="tpp", tag="tpp")
                nc.tensor.matmul(aop, aoT[0], ident_sb, is_transpose=True,
                                 start=True, stop=True)
                ao_sb = work.tile([64, 128], F32, name="ao_sb", tag="ao_sb")
                nc.scalar.activation(ao_sb, aop, AF.Copy)
                c3p = ps_mm.tile([64, 128], F32, name="mmp", tag="mmp")
                nc.tensor.matmul(c3p, w3t_sb, ao_sb, start=True, stop=True)
                t2 = work.tile([64, 128], F32, name="t2", tag="t2")
                nc.vector.scalar_tensor_tensor(t2, c3p, c3s_sb[:, 0:1],
                                               res_sb[:, n * 128:(n + 1) * 128],
                                               ALU.mult, ALU.add)
                on = work.tile([64, 128], F32, name="on", tag="on")
                nc.scalar.activation(on, t2, AF.Relu, bias=c3b_sb[:, 0:1], scale=1.0)
                nc.sync.dma_start(out=outn[:, n * 128:(n + 1) * 128], in_=on)
    return nc


def _run(build_fn, in_maps):
    nc = bacc.Bacc()
    build_fn(nc)
    nc.finalize()
    import os
    tr = bool(int(os.environ.get("BASS_KERNEL_TRACE", "0")))
    r = run_bass_kernel_spmd(nc, in_maps, list(range(NCORES)), trace=tr)
    if tr and r.exec_time_ns is not None:
        print("launch exec_time_ns:", r.exec_time_ns)
    return r.results


def kernel(x, conv1_w, bn1, kqv_w_h, kqv_bn_h, logits_bn_h, rel_enc_h,
           kqv_w_w, kqv_bn_w, logits_bn_w, rel_enc_w, conv3_w, bn3):
    x = np.asarray(x, np.float32)
    B, C, H, W = x.shape  # 1, 64, 256, 128
    wsh = W // NCORES     # 16
    hsh = H // NCORES     # 32

    s1c, b1c = _bn_sb(np.asarray(bn1))
    s3c, b3c = _bn_sb(np.asarray(bn3))
    fh = _axial_folds(np.asarray(kqv_w_h), np.asarray(kqv_bn_h),
                      np.asarray(logits_bn_h))
    fw = _axial_folds(np.asarray(kqv_w_w), np.asarray(kqv_bn_w),
                      np.asarray(logits_bn_w))
    (wkqvt_h, ks4_h, kb4_h, qs4_h, qb4_h, vs_h, vb_h, s2_h, s31_h) = fh
    (wkqvt_w, ks4_w, kb4_w, qs4_w, qb4_w, vs_w, vb_w, s2_w, s31_w) = fw
    qe_h, ke_h, ve_h = _enc_arrays(np.asarray(rel_enc_h, np.float32), 256)
    qe_w, ke_w, ve_w = _enc_arrays(np.asarray(rel_enc_w, np.float32), 128)
    ident = np.eye(128, dtype=np.float32)

    shared1 = dict(
        w1t=np.ascontiguousarray(np.asarray(conv1_w, np.float32).T),
        c1s=s1c.reshape(64, 1), c1b=b1c.reshape(64, 1),
        wkqvt=wkqvt_h, ks4=ks4_h, kb4=kb4_h, qs4=qs4_h, qb4=qb4_h,
        vsc=vs_h, vbc=vb_h,
        qenc=qe_h, kenc=ke_h, venc=_bf16(ve_h), ident=ident)
    in_maps1 = []
    for c in range(NCORES):
        xs = np.ascontiguousarray(
            x[0, :, :, c * wsh:(c + 1) * wsh]).reshape(64, 256 * wsh)
        m = dict(shared1)
        m["xin"] = xs
        in_maps1.append(m)
    res1 = _run(lambda nc: _build_axial(nc, 256, wsh, True, s2_h, s31_h), in_maps1)

    # gather oHT [n(w), 256(H), 64(c)] per core -> oH [64, 256, 128]
    oH = np.empty((64, 256, 128), np.float32)
    for c in range(NCORES):
        t = res1[c]["oHT"]  # [wsh, 256, 64]
        oH[:, :, c * wsh:(c + 1) * wsh] = t.transpose(2, 1, 0)

    shared2 = dict(
        w3t=np.ascontiguousarray(np.asarray(conv3_w, np.float32).T),
        c3s=s3c.reshape(64, 1), c3b=b3c.reshape(64, 1),
        wkqvt=wkqvt_w, ks4=ks4_w, kb4=kb4_w, qs4=qs4_w, qb4=qb4_w,
        vsc=vs_w, vbc=vb_w,
        qenc=qe_w, kenc=ke_w, venc=_bf16(ve_w), ident=ident)
    in_maps2 = []
    for c in range(NCORES):
        sl = slice(c * hsh, (c + 1) * hsh)
        m = dict(shared2)
        m["xin"] = np.ascontiguousarray(oH[:, sl, :]).reshape(64, hsh * 128)
        m["res"] = np.ascontiguousarray(x[0, :, sl, :]).reshape(64, hsh * 128)
        in_maps2.append(m)
    res2 = _run(lambda nc: _build_axial(nc, 128, hsh, False, s2_w, s31_w), in_maps2)

    out = np.empty((1, 64, 256, 128), np.float32)
    for c in range(NCORES):
        out[0, :, c * hsh:(c + 1) * hsh, :] = res2[c]["outn"].reshape(64, hsh, 128)
    return out


# revision 21
# speedup vs baseline: 2644.6237x; 1.0245x over previous
"""Trainium2 Bass kernel for nn_AxialBottleneck (conv1x1+BN+relu -> axial-attn(H)
-> axial-attn(W) -> relu -> conv1x1+BN -> relu(+residual)).

Self-contained: accepts FULL inputs, shards across 8 NeuronCores internally
(axial-H sharded over W, axial-W sharded over H; two SPMD launches with a host
reshard between), returns the FULL output.
"""

import sys
from contextlib import ExitStack

import numpy as np

for _p in ("/opt/trn_rl_repo",):
    if _p not in sys.path:
        sys.path.insert(0, _p)

import concourse.bass as bass
import concourse.bacc as bacc
import concourse.mybir as mybir
import concourse.tile as tile
from concourse.bass_utils import run_bass_kernel_spmd

def _bf16(a):
    import ml_dtypes
    return a.astype(ml_dtypes.bfloat16)


EPS = 1e-5
HEADS = 8
NCORES = 8
F32 = mybir.dt.float32
BF16 = mybir.dt.bfloat16
AF = mybir.ActivationFunctionType
ALU = mybir.AluOpType

# channel permutation: old kqv row 16h+i -> new row (k: 8i+h | q: 32+8(i-4)+h | v: 64+8(i-8)+h)
def _old_of_new():
    o = np.zeros(128, np.int64)
    for old in range(128):
        h, i = old // 16, old % 16
        if i < 4:
            new = 8 * i + h
        elif i < 8:
            new = 32 + 8 * (i - 4) + h
        else:
            new = 64 + 8 * (i - 8) + h
        o[new] = old
    return o


def _bn_sb(p):
    g, b, m, v = p.astype(np.float64)
    s = g / np.sqrt(v + EPS)
    return (s).astype(np.float32), (b - m * s).astype(np.float32)


def _enc_arrays(renc, L):
    # o[a, b] = b - a + L - 1
    o = np.arange(L)[None, :] - np.arange(L)[:, None] + L - 1
    qencS = renc[0:4][:, o]   # [4, x, y] = renc[d, y-x+L-1]
    kencST = renc[4:8][:, o]  # [4, y, x] = renc[4+d, x-y+L-1]
    vencS = renc[8:16][:, o]  # [8, x, y] = renc[8+d, y-x+L-1]
    return (np.ascontiguousarray(qencS, np.float32),
            np.ascontiguousarray(kencST, np.float32),
            np.ascontiguousarray(vencS, np.float32))


def _axial_folds(kqv_w, kqv_bn, logits_bn):
    """Permuted lhsT weight [64,128] (cols: k per-head 4h.., q 32+4h.., v 64+8h+d),
    per-head ACT scale/bias [4,8] for k and q, v scale/bias [64,1], s2[8], s31[8]."""
    sBN, bBN = _bn_sb(kqv_bn)          # [128]
    sL, _ = _bn_sb(logits_bn)          # [24]; bias part drops in softmax
    s1, s2, s3 = sL[0:8], sL[8:16], sL[16:24]
    old_k = np.array([16 * h + d for h in range(HEADS) for d in range(4)])
    old_q = np.array([16 * h + 4 + d for h in range(HEADS) for d in range(4)])
    old_v = np.array([16 * h + 8 + d for h in range(HEADS) for d in range(8)])
    order = np.concatenate([old_k, old_q, old_v])
    Wp = kqv_w[order]                  # [128, 64]
    ks4 = (sBN[old_k] * np.repeat(s1, 4)).reshape(8, 4).T.copy()   # [4, 8]
    kb4 = (bBN[old_k] * np.repeat(s1, 4)).reshape(8, 4).T.copy()
    qs4 = sBN[old_q].reshape(8, 4).T.copy()
    qb4 = bBN[old_q].reshape(8, 4).T.copy()
    vs = sBN[old_v].reshape(64, 1)
    vb = bBN[old_v].reshape(64, 1)
    return (np.ascontiguousarray(Wp.T, np.float32),
            ks4.astype(np.float32), kb4.astype(np.float32),
            qs4.astype(np.float32), qb4.astype(np.float32),
            vs.astype(np.float32), vb.astype(np.float32),
            s2.astype(np.float32), (s3 / s1).astype(np.float32))


def _build_axial(nc, L, n_per, first, s2, s31):
    """first=True: conv1 + axial-H (xin [64, 256*wsh] W-sliced; out oHT
    [n_per, 256, 64]). first=False: axial-W + relu + conv3 + bn3 + residual
    relu (xin/res [64, n_per*128] H-sliced; out outn [64, n_per*128])."""
    nyt = L // 128
    nxt = L // 128
    if first:
        wsh = 128 // NCORES
        xin = nc.dram_tensor("xin", [64, 256 * wsh], F32, kind="ExternalInput")
        w1t = nc.dram_tensor("w1t", [64, 64], F32, kind="ExternalInput")
        c1s = nc.dram_tensor("c1s", [64, 1], F32, kind="ExternalInput")
        c1b = nc.dram_tensor("c1b", [64, 1], F32, kind="ExternalInput")
        oHT = nc.dram_tensor("oHT", [n_per, 256, 64], F32, kind="ExternalOutput")
    else:
        xin = nc.dram_tensor("xin", [64, n_per * 128], F32, kind="ExternalInput")
        res = nc.dram_tensor("res", [64, n_per * 128], F32, kind="ExternalInput")
        w3t = nc.dram_tensor("w3t", [64, 64], F32, kind="ExternalInput")
        c3s = nc.dram_tensor("c3s", [64, 1], F32, kind="ExternalInput")
        c3b = nc.dram_tensor("c3b", [64, 1], F32, kind="ExternalInput")
        outn = nc.dram_tensor("outn", [64, n_per * 128], F32, kind="ExternalOutput")
    wkqvt = nc.dram_tensor("wkqvt", [64, 128], F32, kind="ExternalInput")
    ks4 = nc.dram_tensor("ks4", [4, 8], F32, kind="ExternalInput")
    kb4 = nc.dram_tensor("kb4", [4, 8], F32, kind="ExternalInput")
    qs4 = nc.dram_tensor("qs4", [4, 8], F32, kind="ExternalInput")
    qb4 = nc.dram_tensor("qb4", [4, 8], F32, kind="ExternalInput")
    vsc = nc.dram_tensor("vsc", [64, 1], F32, kind="ExternalInput")
    vbc = nc.dram_tensor("vbc", [64, 1], F32, kind="ExternalInput")
    qenc = nc.dram_tensor("qenc", [4, L, L], F32, kind="ExternalInput")
    kenc = nc.dram_tensor("kenc", [4, L, L], BF16, kind="ExternalInput")
    venc = nc.dram_tensor("venc", [8, L, L], BF16, kind="ExternalInput")
    ident = nc.dram_tensor("ident", [128, 128], F32, kind="ExternalInput")

    with ExitStack() as ctx:
        tc = ctx.enter_context(tile.TileContext(nc))
        singles = ctx.enter_context(tc.tile_pool(name="singles", bufs=1))
        work = ctx.enter_context(tc.tile_pool(name="work", bufs=3))
        wide = ctx.enter_context(tc.tile_pool(name="wide", bufs=2))
        ps_mm = ctx.enter_context(tc.tile_pool(name="ps_mm", bufs=2, space="PSUM"))
        ps_lt = ctx.enter_context(tc.tile_pool(name="ps_lt", bufs=2, space="PSUM"))
        ps_el = ctx.enter_context(tc.tile_pool(name="ps_el", bufs=2, space="PSUM"))
        ps_at = ctx.enter_context(tc.tile_pool(name="ps_at", bufs=1, space="PSUM"))
        ps_tp = ctx.enter_context(tc.tile_pool(name="ps_tp", bufs=1, space="PSUM"))

        def load(name, dram, shape):
            t = singles.tile(shape, F32, name=name, tag=name)
            nc.sync.dma_start(out=t, in_=dram[:, :] if len(shape) == 2 else dram)
            return t

        ident_sb = load("ident", ident, [128, 128])
        identB = singles.tile([128, 128], BF16, name="identB", tag="identB")
        nc.scalar.activation(identB, ident_sb, AF.Copy)
        wkqvt_d = load("wkqvt", wkqvt, [64, 128])
        wkqvt_sb = singles.tile([64, 128], F32, name="wkqvt_c", tag="wkqvt_c")
        nc.vector.tensor_scalar(wkqvt_sb, wkqvt_d, 1.0, None, ALU.mult)
        ks4_sb = load("ks4", ks4, [4, 8])
        kb4_sb = load("kb4", kb4, [4, 8])
        qs4_sb = load("qs4", qs4, [4, 8])
        qb4_sb = load("qb4", qb4, [4, 8])
        vsc_sb = load("vsc", vsc, [64, 1])
        vbc_sb = load("vbc", vbc, [64, 1])
        qenc_sb = {}
        kenc_sb = {}
        venc_sb = {}
        for d in range(4):
            for xt in range(nxt):
                t = singles.tile([128, L], F32, name=f"qe{d}_{xt}", tag=f"qe{d}_{xt}")
                nc.sync.dma_start(out=t, in_=qenc[d, xt * 128:(xt + 1) * 128, :])
                qenc_sb[(d, xt)] = t
            for yt in range(nyt):
                t = singles.tile([128, L], BF16, name=f"ke{d}_{yt}", tag=f"ke{d}_{yt}")
                nc.sync.dma_start(out=t, in_=kenc[d, yt * 128:(yt + 1) * 128, :])
                kenc_sb[(d, yt)] = t
        for d in range(8):
            for xt in range(nxt):
                t = singles.tile([128, L], BF16, name=f"ve{d}_{xt}", tag=f"ve{d}_{xt}")
                nc.sync.dma_start(out=t, in_=venc[d, xt * 128:(xt + 1) * 128, :])
                venc_sb[(d, xt)] = t

        if first:
            w1t_d = load("w1t", w1t, [64, 64])
            w1t_sb = singles.tile([64, 64], F32, name="w1t_c", tag="w1t_c")
            nc.vector.tensor_scalar(w1t_sb, w1t_d, 1.0, None, ALU.mult)
            c1s_sb = load("c1s", c1s, [64, 1])
            c1b_sb = load("c1b", c1b, [64, 1])
            wsh = 128 // NCORES
            npx = 256 * wsh
            xin_sb = load("xin", xin, [64, npx])
            conv_sb = singles.tile([64, npx], F32, name="conv", tag="conv")
            for ck in range(npx // 512):
                cp = ps_mm.tile([64, 512], F32, name="mmp", tag="mmp")
                nc.tensor.matmul(cp, w1t_sb, xin_sb[:, ck * 512:(ck + 1) * 512],
                                 start=True, stop=True)
                nc.scalar.activation(conv_sb[:, ck * 512:(ck + 1) * 512], cp,
                                     AF.Relu, bias=c1b_sb[:, 0:1], scale=c1s_sb[:, 0:1])
        else:
            w3t_d = load("w3t", w3t, [64, 64])
            w3t_sb = singles.tile([64, 64], F32, name="w3t_c", tag="w3t_c")
            nc.vector.tensor_scalar(w3t_sb, w3t_d, 1.0, None, ALU.mult)
            c3s_sb = load("c3s", c3s, [64, 1])
            c3b_sb = load("c3b", c3b, [64, 1])
            xin_sb = load("xin", xin, [64, n_per * 128])
            res_sb = load("res", res, [64, n_per * 128])

        for n in range(n_per):
            if first:
                wsh = 128 // NCORES
                xf = conv_sb[:, n:256 * wsh:wsh]
            else:
                xf = xin_sb[:, n * 128:(n + 1) * 128]
            # per-head k, q tiles [4, L]; v tile [64, L]
            kh, qh = [], []
            for h in range(HEADS):
                kp = ps_mm.tile([4, L], F32, name="mmp", tag="mmp")
                nc.tensor.matmul(kp, wkqvt_sb[:, 4 * h:4 * h + 4], xf,
                                 start=True, stop=True)
                kt = work.tile([4, L], F32, name="kh", tag="kh", bufs=10)
                nc.scalar.activation(kt, kp, AF.Identity,
                                     bias=kb4_sb[:, h:h + 1], scale=ks4_sb[:, h:h + 1])
                kh.append(kt)
                qp = ps_mm.tile([4, L], F32, name="mmp", tag="mmp")
                nc.tensor.matmul(qp, wkqvt_sb[:, 32 + 4 * h:32 + 4 * h + 4], xf,
                                 start=True, stop=True)
                qt = work.tile([4, L], F32, name="qh", tag="qh", bufs=10)
                nc.scalar.activation(qt, qp, AF.Identity,
                                     bias=qb4_sb[:, h:h + 1], scale=qs4_sb[:, h:h + 1])
                qh.append(qt)
            vp = ps_mm.tile([64, L], F32, name="mmp", tag="mmp")
            nc.tensor.matmul(vp, wkqvt_sb[:, 64:128], xf, start=True, stop=True)
            vt = wide.tile([64, L], F32, name="vt", tag="vt", bufs=2)
            nc.scalar.activation(vt, vp, AF.Identity,
                                 bias=vbc_sb[:, 0:1], scale=vsc_sb[:, 0:1])
            vT = []
            for yt in range(nyt):
                tp = ps_tp.tile([128, 64], F32, name="tpp", tag="tpp")
                nc.tensor.matmul(tp, vt[:, yt * 128:(yt + 1) * 128],
                                 ident_sb[0:64, 0:64], is_transpose=True,
                                 start=True, stop=True)
                ts = work.tile([128, 64], F32, name="vT", tag="vT", bufs=4)
                nc.scalar.activation(ts, tp, AF.Copy)
                vT.append(ts)
            aoT = []
            for xt in range(nxt):
                aoT.append(work.tile([128, 64], F32, name=f"aoT{xt}", tag=f"aoT{xt}"))
            for h in range(HEADS):
                ksc, qsc = [], []
                for yt in range(nyt):
                    tp = ps_tp.tile([128, 4], F32, name="tpp", tag="tpp")
                    nc.tensor.matmul(tp, kh[h][:, yt * 128:(yt + 1) * 128],
                                     ident_sb[0:4, 0:4], is_transpose=True,
                                     start=True, stop=True)
                    t = work.tile([128, 4], F32, name="ksc", tag="ksc", bufs=4)
                    nc.scalar.activation(t, tp, AF.Copy, scale=float(s31[h]))
                    ksc.append(t)
                for xt in range(nxt):
                    tp = ps_tp.tile([128, 4], F32, name="tpp", tag="tpp")
                    nc.tensor.matmul(tp, qh[h][:, xt * 128:(xt + 1) * 128],
                                     ident_sb[0:4, 0:4], is_transpose=True,
                                     start=True, stop=True)
                    t = work.tile([128, 4], F32, name="qsc", tag="qsc", bufs=4)
                    nc.scalar.activation(t, tp, AF.Copy, scale=float(s2[h]))
                    qsc.append(t)
                # qr natural [x, y]
                qr = []
                for xt in range(nxt):
                    t = wide.tile([128, L], F32, name="qr", tag="qr")
                    nc.vector.tensor_scalar(t, qenc_sb[(0, xt)], qsc[xt][:, 0:1],
                                            None, ALU.mult)
                    for d in range(1, 4):
                        nc.vector.scalar_tensor_tensor(
                            t, qenc_sb[(d, xt)], qsc[xt][:, d:d + 1], t,
                            ALU.mult, ALU.add)
                    qr.append(t)
                # logits^T [y, x] in PSUM: qk + qr^T + kr
                LT = []
                for yt in range(nyt):
                    lt = ps_lt.tile([128, L], F32, name="lt", tag="lt")
                    nc.tensor.matmul(lt, kh[h][:, yt * 128:(yt + 1) * 128],
                                     qh[h], start=True, stop=False)
                    for xt in range(nxt):
                        nc.tensor.matmul(lt[:, xt * 128:(xt + 1) * 128],
                                         qr[xt][:, yt * 128:(yt + 1) * 128],
                                         ident_sb, is_transpose=True,
                                         start=False, stop=(xt == nxt - 1))
                    kra = wide.tile([128, L], BF16, name="kra", tag="kra")
                    nc.vector.tensor_scalar(kra, kenc_sb[(0, yt)],
                                            ksc[yt][:, 0:1], None, ALU.mult)
                    for d in range(1, 4):
                        nc.vector.scalar_tensor_tensor(
                            kra, kenc_sb[(d, yt)], ksc[yt][:, d:d + 1], kra,
                            ALU.mult, ALU.add)
                    nc.vector.tensor_tensor(out=lt, in0=lt, in1=kra, op=ALU.add)
                    LT.append(lt)
                expLT = []
                for yt in range(nyt):
                    e = wide.tile([128, L], F32, name="explt", tag="explt", bufs=4)
                    nc.scalar.activation(e, LT[yt], AF.Exp)
                    expLT.append(e)
                arhs = []
                for yt in range(nyt):
                    r = work.tile([128, 9], F32, name="arhs", tag="arhs", bufs=4)
                    nc.vector.tensor_scalar(r[:, 0:8], vT[yt][:, 8 * h:8 * h + 8],
                                            1.0, None, ALU.mult)
                    nc.vector.memset(r[:, 8:9], 1.0)
                    arhs.append(r)
                at = ps_at.tile([128, 9 * nxt], F32, name="at", tag="at")
                for xh in range(nxt):
                    for yt in range(nyt):
                        nc.tensor.matmul(at[:, xh * 9:xh * 9 + 9],
                                         expLT[yt][:, xh * 128:(xh + 1) * 128],
                                         arhs[yt], start=(yt == 0),
                                         stop=(yt == nyt - 1))
                for xh in range(nxt):
                    ep = ps_el.tile([128, L], F32, name="expl", tag="expl")
                    for yt in range(nyt):
                        nc.tensor.matmul(ep[:, yt * 128:(yt + 1) * 128],
                                         expLT[yt][:, xh * 128:(xh + 1) * 128],
                                         ident_sb, is_transpose=True,
                                         start=True, stop=True)
                    epb = wide.tile([128, L], BF16, name="epb", tag="epb")
                    nc.scalar.activation(epb, ep, AF.Copy)
                    aenc = work.tile([128, 8], F32, name="aenc", tag="aenc")
                    scratch = wide.tile([128, L], BF16, name="scratch", tag="scratch")
                    for d in range(8):
                        nc.vector.scalar_tensor_tensor(
                            scratch, venc_sb[(d, xh)], 1.0, epb,
                            ALU.mult, ALU.mult,
                            accum_out=aenc[:, d:d + 1])
                    rc = work.tile([128, 1], F32, name="rc", tag="rc")
                    nc.vector.reciprocal(rc, at[:, xh * 9 + 8:xh * 9 + 9])
                    tsum = work.tile([128, 8], F32, name="tsum", tag="tsum")
                    nc.vector.tensor_tensor(out=tsum, in0=at[:, xh * 9:xh * 9 + 8],
                                            in1=aenc, op=ALU.add)
                    if first:
                        nc.vector.tensor_scalar(aoT[xh][:, 8 * h:8 * h + 8], tsum,
                                                rc[:, 0:1], None, ALU.mult)
                    else:
                        nc.vector.tensor_scalar(aoT[xh][:, 8 * h:8 * h + 8], tsum,
                                                rc[:, 0:1], 0.0, ALU.mult, ALU.max)
            if first:
                for xt in range(nxt):
                    nc.sync.dma_start(out=oHT[n, xt * 128:(xt + 1) * 128, :],
                                      in_=aoT[xt])
            else:
                aop = ps_tp.tile([64, 128], F32, name="tpp", tag="tpp")
                nc.tensor.matmul(aop, aoT[0], ident_sb, is_transpose=True,
                                 start=True, stop=True)
                ao_sb = work.tile([64, 128], F32, name="ao_sb", tag="ao_sb")
                nc.scalar.activation(ao_sb, aop, AF.Copy)
                c3p = ps_mm.tile([64, 128], F32, name="mmp", tag="mmp")
                nc.tensor.matmul(c3p, w3t_sb, ao_sb, start=True, stop=True)
                t2 = work.tile([64, 128], F32, name="t2", tag="t2")
                nc.vector.scalar_tensor_tensor(t2, c3p, c3s_sb[:, 0:1],
                                               res_sb[:, n * 128:(n + 1) * 128],
                                               ALU.mult, ALU.add)
                on = work.tile([64, 128], F32, name="on", tag="on")
                nc.scalar.activation(on, t2, AF.Relu, bias=c3b_sb[:, 0:1], scale=1.0)
                nc.sync.dma_start(out=outn[:, n * 128:(n + 1) * 128], in_=on)
    return nc


def _run(build_fn, in_maps):
    nc = bacc.Bacc()
    build_fn(nc)
    nc.finalize()
    import os
    tr = bool(int(os.environ.get("BASS_KERNEL_TRACE", "0")))
    r = run_bass_kernel_spmd(nc, in_maps, list(range(NCORES)), trace=tr)
    if tr and r.exec_time_ns is not None:
        print("launch exec_time_ns:", r.exec_time_ns)
    return r.results


def kernel(x, conv1_w, bn1, kqv_w_h, kqv_bn_h, logits_bn_h, rel_enc_h,
           kqv_w_w, kqv_bn_w, logits_bn_w, rel_enc_w, conv3_w, bn3):
    x = np.asarray(x, np.float32)
    B, C, H, W = x.shape  # 1, 64, 256, 128
    wsh = W // NCORES     # 16
    hsh = H // NCORES     # 32

    s1c, b1c = _bn_sb(np.asarray(bn1))
    s3c, b3c = _bn_sb(np.asarray(bn3))
    fh = _axial_folds(np.asarray(kqv_w_h), np.asarray(kqv_bn_h),
                      np.asarray(logits_bn_h))
    fw = _axial_folds(np.asarray(kqv_w_w), np.asarray(kqv_bn_w),
                      np.asarray(logits_bn_w))
    (wkqvt_h, ks4_h, kb4_h, qs4_h, qb4_h, vs_h, vb_h, s2_h, s31_h) = fh
    (wkqvt_w, ks4_w, kb4_w, qs4_w, qb4_w, vs_w, vb_w, s2_w, s31_w) = fw
    qe_h, ke_h, ve_h = _enc_arrays(np.asarray(rel_enc_h, np.float32), 256)
    qe_w, ke_w, ve_w = _enc_arrays(np.asarray(rel_enc_w, np.float32), 128)
    ident = np.eye(128, dtype=np.float32)

    shared1 = dict(
        w1t=np.ascontiguousarray(np.asarray(conv1_w, np.float32).T),
        c1s=s1c.reshape(64, 1), c1b=b1c.reshape(64, 1),
        wkqvt=wkqvt_h, ks4=ks4_h, kb4=kb4_h, qs4=qs4_h, qb4=qb4_h,
        vsc=vs_h, vbc=vb_h,
        qenc=qe_h, kenc=_bf16(ke_h), venc=_bf16(ve_h), ident=ident)
    in_maps1 = []
    for c in range(NCORES):
        xs = np.ascontiguousarray(
            x[0, :, :, c * wsh:(c + 1) * wsh]).reshape(64, 256 * wsh)
        m = dict(shared1)
        m["xin"] = xs
        in_maps1.append(m)
    res1 = _run(lambda nc: _build_axial(nc, 256, wsh, True, s2_h, s31_h), in_maps1)

    # gather oHT [n(w), 256(H), 64(c)] per core -> oH [64, 256, 128]
    oH = np.empty((64, 256, 128), np.float32)
    for c in range(NCORES):
        t = res1[c]["oHT"]  # [wsh, 256, 64]
        oH[:, :, c * wsh:(c + 1) * wsh] = t.transpose(2, 1, 0)

    shared2 = dict(
        w3t=np.ascontiguousarray(np.asarray(conv3_w, np.float32).T),
        c3s=s3c.reshape(64, 1), c3b=b3c.reshape(64, 1),
        wkqvt=wkqvt_w, ks4=ks4_w, kb4=kb4_w, qs4=qs4_w, qb4=qb4_w,
        vsc=vs_w, vbc=vb_w,
        qenc=qe_w, kenc=_bf16(ke_w), venc=_bf16(ve_w), ident=ident)
    in_maps2 = []
    for c in range(NCORES):
        sl = slice(c * hsh, (c + 1) * hsh)
        m = dict(shared2)
        m["xin"] = np.ascontiguousarray(oH[:, sl, :]).reshape(64, hsh * 128)
        m["res"] = np.ascontiguousarray(x[0, :, sl, :]).reshape(64, hsh * 128)
        in_maps2.append(m)
    res2 = _run(lambda nc: _build_axial(nc, 128, hsh, False, s2_w, s31_w), in_maps2)

    out = np.empty((1, 64, 256, 128), np.float32)
    for c in range(NCORES):
        out[0, :, c * hsh:(c + 1) * hsh, :] = res2[c]["outn"].reshape(64, hsh, 128)
    return out


# revision 22
# speedup vs baseline: 2665.1767x; 1.0078x over previous
"""Trainium2 Bass kernel for nn_AxialBottleneck (conv1x1+BN+relu -> axial-attn(H)
-> axial-attn(W) -> relu -> conv1x1+BN -> relu(+residual)).

Self-contained: accepts FULL inputs, shards across 8 NeuronCores internally
(axial-H sharded over W, axial-W sharded over H; two SPMD launches with a host
reshard between), returns the FULL output.
"""

import sys
from contextlib import ExitStack

import numpy as np

for _p in ("/opt/trn_rl_repo",):
    if _p not in sys.path:
        sys.path.insert(0, _p)

import concourse.bass as bass
import concourse.bacc as bacc
import concourse.mybir as mybir
import concourse.tile as tile
from concourse.bass_utils import run_bass_kernel_spmd

def _bf16(a):
    import ml_dtypes
    return a.astype(ml_dtypes.bfloat16)


EPS = 1e-5
HEADS = 8
NCORES = 8
F32 = mybir.dt.float32
BF16 = mybir.dt.bfloat16
AF = mybir.ActivationFunctionType
ALU = mybir.AluOpType

# channel permutation: old kqv row 16h+i -> new row (k: 8i+h | q: 32+8(i-4)+h | v: 64+8(i-8)+h)
def _old_of_new():
    o = np.zeros(128, np.int64)
    for old in range(128):
        h, i = old // 16, old % 16
        if i < 4:
            new = 8 * i + h
        elif i < 8:
            new = 32 + 8 * (i - 4) + h
        else:
            new = 64 + 8 * (i - 8) + h
        o[new] = old
    return o


def _bn_sb(p):
    g, b, m, v = p.astype(np.float64)
    s = g / np.sqrt(v + EPS)
    return (s).astype(np.float32), (b - m * s).astype(np.float32)


def _enc_arrays(renc, L):
    # o[a, b] = b - a + L - 1
    o = np.arange(L)[None, :] - np.arange(L)[:, None] + L - 1
    qencS = renc[0:4][:, o]   # [4, x, y] = renc[d, y-x+L-1]
    kencST = renc[4:8][:, o]  # [4, y, x] = renc[4+d, x-y+L-1]
    vencS = renc[8:16][:, o]  # [8, x, y] = renc[8+d, y-x+L-1]
    return (np.ascontiguousarray(qencS, np.float32),
            np.ascontiguousarray(kencST, np.float32),
            np.ascontiguousarray(vencS, np.float32))


def _axial_folds(kqv_w, kqv_bn, logits_bn):
    """Permuted lhsT weight [64,128] (cols: k per-head 4h.., q 32+4h.., v 64+8h+d),
    per-head ACT scale/bias [4,8] for k and q, v scale/bias [64,1], s2[8], s31[8]."""
    sBN, bBN = _bn_sb(kqv_bn)          # [128]
    sL, _ = _bn_sb(logits_bn)          # [24]; bias part drops in softmax
    s1, s2, s3 = sL[0:8], sL[8:16], sL[16:24]
    old_k = np.array([16 * h + d for h in range(HEADS) for d in range(4)])
    old_q = np.array([16 * h + 4 + d for h in range(HEADS) for d in range(4)])
    old_v = np.array([16 * h + 8 + d for h in range(HEADS) for d in range(8)])
    order = np.concatenate([old_k, old_q, old_v])
    Wp = kqv_w[order]                  # [128, 64]
    ks4 = (sBN[old_k] * np.repeat(s1, 4)).reshape(8, 4).T.copy()   # [4, 8]
    kb4 = (bBN[old_k] * np.repeat(s1, 4)).reshape(8, 4).T.copy()
    qs4 = sBN[old_q].reshape(8, 4).T.copy()
    qb4 = bBN[old_q].reshape(8, 4).T.copy()
    vs = sBN[old_v].reshape(64, 1)
    vb = bBN[old_v].reshape(64, 1)
    return (np.ascontiguousarray(Wp.T, np.float32),
            ks4.astype(np.float32), kb4.astype(np.float32),
            qs4.astype(np.float32), qb4.astype(np.float32),
            vs.astype(np.float32), vb.astype(np.float32),
            s2.astype(np.float32), (s3 / s1).astype(np.float32))


def _build_axial(nc, L, n_per, first, s2, s31):
    """first=True: conv1 + axial-H (xin [64, 256*wsh] W-sliced; out oHT
    [n_per, 256, 64]). first=False: axial-W + relu + conv3 + bn3 + residual
    relu (xin/res [64, n_per*128] H-sliced; out outn [64, n_per*128])."""
    nyt = L // 128
    nxt = L // 128
    if first:
        wsh = 128 // NCORES
        xin = nc.dram_tensor("xin", [64, 256 * wsh], F32, kind="ExternalInput")
        w1t = nc.dram_tensor("w1t", [64, 64], F32, kind="ExternalInput")
        c1s = nc.dram_tensor("c1s", [64, 1], F32, kind="ExternalInput")
        c1b = nc.dram_tensor("c1b", [64, 1], F32, kind="ExternalInput")
        oHT = nc.dram_tensor("oHT", [n_per, 256, 64], F32, kind="ExternalOutput")
    else:
        xin = nc.dram_tensor("xin", [64, n_per * 128], F32, kind="ExternalInput")
        res = nc.dram_tensor("res", [64, n_per * 128], F32, kind="ExternalInput")
        w3t = nc.dram_tensor("w3t", [64, 64], F32, kind="ExternalInput")
        c3s = nc.dram_tensor("c3s", [64, 1], F32, kind="ExternalInput")
        c3b = nc.dram_tensor("c3b", [64, 1], F32, kind="ExternalInput")
        outn = nc.dram_tensor("outn", [64, n_per * 128], F32, kind="ExternalOutput")
    wkqvt = nc.dram_tensor("wkqvt", [64, 128], F32, kind="ExternalInput")
    ks4 = nc.dram_tensor("ks4", [4, 8], F32, kind="ExternalInput")
    kb4 = nc.dram_tensor("kb4", [4, 8], F32, kind="ExternalInput")
    qs4 = nc.dram_tensor("qs4", [4, 8], F32, kind="ExternalInput")
    qb4 = nc.dram_tensor("qb4", [4, 8], F32, kind="ExternalInput")
    vsc = nc.dram_tensor("vsc", [64, 1], F32, kind="ExternalInput")
    vbc = nc.dram_tensor("vbc", [64, 1], F32, kind="ExternalInput")
    qenc = nc.dram_tensor("qenc", [4, L, L], F32, kind="ExternalInput")
    kenc = nc.dram_tensor("kenc", [4, L, L], BF16, kind="ExternalInput")
    venc = nc.dram_tensor("venc", [8, L, L], BF16, kind="ExternalInput")
    ident = nc.dram_tensor("ident", [128, 128], F32, kind="ExternalInput")

    with ExitStack() as ctx:
        tc = ctx.enter_context(tile.TileContext(nc))
        singles = ctx.enter_context(tc.tile_pool(name="singles", bufs=1))
        work = ctx.enter_context(tc.tile_pool(name="work", bufs=3))
        wide = ctx.enter_context(tc.tile_pool(name="wide", bufs=2))
        ps_mm = ctx.enter_context(tc.tile_pool(name="ps_mm", bufs=2, space="PSUM"))
        ps_lt = ctx.enter_context(tc.tile_pool(name="ps_lt", bufs=2, space="PSUM"))
        ps_el = ctx.enter_context(tc.tile_pool(name="ps_el", bufs=2, space="PSUM"))
        ps_at = ctx.enter_context(tc.tile_pool(name="ps_at", bufs=1, space="PSUM"))
        ps_tp = ctx.enter_context(tc.tile_pool(name="ps_tp", bufs=1, space="PSUM"))

        def load(name, dram, shape):
            t = singles.tile(shape, F32, name=name, tag=name)
            nc.sync.dma_start(out=t, in_=dram[:, :] if len(shape) == 2 else dram)
            return t

        ident_sb = load("ident", ident, [128, 128])
        identB = singles.tile([128, 128], BF16, name="identB", tag="identB")
        nc.scalar.activation(identB, ident_sb, AF.Copy)
        wkqvt_d = load("wkqvt", wkqvt, [64, 128])
        wkqvt_sb = singles.tile([64, 128], F32, name="wkqvt_c", tag="wkqvt_c")
        nc.vector.tensor_scalar(wkqvt_sb, wkqvt_d, 1.0, None, ALU.mult)
        ks4_sb = load("ks4", ks4, [4, 8])
        kb4_sb = load("kb4", kb4, [4, 8])
        qs4_sb = load("qs4", qs4, [4, 8])
        qb4_sb = load("qb4", qb4, [4, 8])
        vsc_sb = load("vsc", vsc, [64, 1])
        vbc_sb = load("vbc", vbc, [64, 1])
        qenc_sb = {}
        kenc_sb = {}
        venc_sb = {}
        for d in range(4):
            for xt in range(nxt):
                t = singles.tile([128, L], F32, name=f"qe{d}_{xt}", tag=f"qe{d}_{xt}")
                nc.sync.dma_start(out=t, in_=qenc[d, xt * 128:(xt + 1) * 128, :])
                qenc_sb[(d, xt)] = t
            for yt in range(nyt):
                t = singles.tile([128, L], BF16, name=f"ke{d}_{yt}", tag=f"ke{d}_{yt}")
                nc.sync.dma_start(out=t, in_=kenc[d, yt * 128:(yt + 1) * 128, :])
                kenc_sb[(d, yt)] = t
        for d in range(8):
            for xt in range(nxt):
                t = singles.tile([128, L], BF16, name=f"ve{d}_{xt}", tag=f"ve{d}_{xt}")
                nc.sync.dma_start(out=t, in_=venc[d, xt * 128:(xt + 1) * 128, :])
                venc_sb[(d, xt)] = t

        if first:
            w1t_d = load("w1t", w1t, [64, 64])
            w1t_sb = singles.tile([64, 64], F32, name="w1t_c", tag="w1t_c")
            nc.vector.tensor_scalar(w1t_sb, w1t_d, 1.0, None, ALU.mult)
            c1s_sb = load("c1s", c1s, [64, 1])
            c1b_sb = load("c1b", c1b, [64, 1])
            wsh = 128 // NCORES
            npx = 256 * wsh
            xin_sb = load("xin", xin, [64, npx])
            conv_sb = singles.tile([64, npx], F32, name="conv", tag="conv")
            for ck in range(npx // 512):
                cp = ps_mm.tile([64, 512], F32, name="mmp", tag="mmp")
                nc.tensor.matmul(cp, w1t_sb, xin_sb[:, ck * 512:(ck + 1) * 512],
                                 start=True, stop=True)
                nc.scalar.activation(conv_sb[:, ck * 512:(ck + 1) * 512], cp,
                                     AF.Relu, bias=c1b_sb[:, 0:1], scale=c1s_sb[:, 0:1])
        else:
            w3t_d = load("w3t", w3t, [64, 64])
            w3t_sb = singles.tile([64, 64], F32, name="w3t_c", tag="w3t_c")
            nc.vector.tensor_scalar(w3t_sb, w3t_d, 1.0, None, ALU.mult)
            c3s_sb = load("c3s", c3s, [64, 1])
            c3b_sb = load("c3b", c3b, [64, 1])
            xin_sb = load("xin", xin, [64, n_per * 128])
            res_sb = load("res", res, [64, n_per * 128])

        for n in range(n_per):
            if first:
                wsh = 128 // NCORES
                xf = conv_sb[:, n:256 * wsh:wsh]
            else:
                xf = xin_sb[:, n * 128:(n + 1) * 128]
            # per-head k, q tiles [4, L]; v tile [64, L]
            kh, qh = [], []
            for h in range(HEADS):
                kp = ps_mm.tile([4, L], F32, name="mmp", tag="mmp")
                nc.tensor.matmul(kp, wkqvt_sb[:, 4 * h:4 * h + 4], xf,
                                 start=True, stop=True)
                kt = work.tile([4, L], F32, name="kh", tag="kh", bufs=10)
                nc.scalar.activation(kt, kp, AF.Identity,
                                     bias=kb4_sb[:, h:h + 1], scale=ks4_sb[:, h:h + 1])
                kh.append(kt)
                qp = ps_mm.tile([4, L], F32, name="mmp", tag="mmp")
                nc.tensor.matmul(qp, wkqvt_sb[:, 32 + 4 * h:32 + 4 * h + 4], xf,
                                 start=True, stop=True)
                qt = work.tile([4, L], F32, name="qh", tag="qh", bufs=10)
                nc.scalar.activation(qt, qp, AF.Identity,
                                     bias=qb4_sb[:, h:h + 1], scale=qs4_sb[:, h:h + 1])
                qh.append(qt)
            vp = ps_mm.tile([64, L], F32, name="mmp", tag="mmp")
            nc.tensor.matmul(vp, wkqvt_sb[:, 64:128], xf, start=True, stop=True)
            vt = wide.tile([64, L], F32, name="vt", tag="vt", bufs=2)
            nc.scalar.activation(vt, vp, AF.Identity,
                                 bias=vbc_sb[:, 0:1], scale=vsc_sb[:, 0:1])
            vT = []
            for yt in range(nyt):
                tp = ps_tp.tile([128, 64], F32, name="tpp", tag="tpp")
                nc.tensor.matmul(tp, vt[:, yt * 128:(yt + 1) * 128],
                                 ident_sb[0:64, 0:64], is_transpose=True,
                                 start=True, stop=True)
                ts = work.tile([128, 72], F32, name="vT", tag="vT", bufs=4)
                ts3 = ts.rearrange("p (h c) -> p h c", c=9)
                nc.scalar.activation(ts3[:, :, 0:8],
                                     tp.rearrange("p (h d) -> p h d", d=8),
                                     AF.Copy)
                nc.vector.memset(ts3[:, :, 8:9], 1.0)
                vT.append(ts)
            aoT = []
            for xt in range(nxt):
                aoT.append(work.tile([128, 64], F32, name=f"aoT{xt}", tag=f"aoT{xt}"))
            for h in range(HEADS):
                ksc, qsc = [], []
                for yt in range(nyt):
                    tp = ps_tp.tile([128, 4], F32, name="tpp", tag="tpp")
                    nc.tensor.matmul(tp, kh[h][:, yt * 128:(yt + 1) * 128],
                                     ident_sb[0:4, 0:4], is_transpose=True,
                                     start=True, stop=True)
                    t = work.tile([128, 4], F32, name="ksc", tag="ksc", bufs=4)
                    nc.scalar.activation(t, tp, AF.Copy, scale=float(s31[h]))
                    ksc.append(t)
                for xt in range(nxt):
                    tp = ps_tp.tile([128, 4], F32, name="tpp", tag="tpp")
                    nc.tensor.matmul(tp, qh[h][:, xt * 128:(xt + 1) * 128],
                                     ident_sb[0:4, 0:4], is_transpose=True,
                                     start=True, stop=True)
                    t = work.tile([128, 4], F32, name="qsc", tag="qsc", bufs=4)
                    nc.scalar.activation(t, tp, AF.Copy, scale=float(s2[h]))
                    qsc.append(t)
                # qr natural [x, y]
                qr = []
                for xt in range(nxt):
                    t = wide.tile([128, L], F32, name="qr", tag="qr")
                    nc.vector.tensor_scalar(t, qenc_sb[(0, xt)], qsc[xt][:, 0:1],
                                            None, ALU.mult)
                    for d in range(1, 4):
                        nc.vector.scalar_tensor_tensor(
                            t, qenc_sb[(d, xt)], qsc[xt][:, d:d + 1], t,
                            ALU.mult, ALU.add)
                    qr.append(t)
                # logits^T [y, x] in PSUM: qk + qr^T + kr
                LT = []
                for yt in range(nyt):
                    lt = ps_lt.tile([128, L], F32, name="lt", tag="lt")
                    nc.tensor.matmul(lt, kh[h][:, yt * 128:(yt + 1) * 128],
                                     qh[h], start=True, stop=False)
                    for xt in range(nxt):
                        nc.tensor.matmul(lt[:, xt * 128:(xt + 1) * 128],
                                         qr[xt][:, yt * 128:(yt + 1) * 128],
                                         ident_sb, is_transpose=True,
                                         start=False, stop=(xt == nxt - 1))
                    kra = wide.tile([128, L], BF16, name="kra", tag="kra")
                    nc.vector.tensor_scalar(kra, kenc_sb[(0, yt)],
                                            ksc[yt][:, 0:1], None, ALU.mult)
                    for d in range(1, 4):
                        nc.vector.scalar_tensor_tensor(
                            kra, kenc_sb[(d, yt)], ksc[yt][:, d:d + 1], kra,
                            ALU.mult, ALU.add)
                    nc.vector.tensor_tensor(out=lt, in0=lt, in1=kra, op=ALU.add)
                    LT.append(lt)
                expLT = []
                for yt in range(nyt):
                    e = wide.tile([128, L], F32, name="explt", tag="explt", bufs=4)
                    nc.scalar.activation(e, LT[yt], AF.Exp)
                    expLT.append(e)
                at = ps_at.tile([128, 9 * nxt], F32, name="at", tag="at")
                for xh in range(nxt):
                    for yt in range(nyt):
                        nc.tensor.matmul(at[:, xh * 9:xh * 9 + 9],
                                         expLT[yt][:, xh * 128:(xh + 1) * 128],
                                         vT[yt][:, 9 * h:9 * h + 9],
                                         start=(yt == 0),
                                         stop=(yt == nyt - 1))
                for xh in range(nxt):
                    ep = ps_el.tile([128, L], F32, name="expl", tag="expl")
                    for yt in range(nyt):
                        nc.tensor.matmul(ep[:, yt * 128:(yt + 1) * 128],
                                         expLT[yt][:, xh * 128:(xh + 1) * 128],
                                         ident_sb, is_transpose=True,
                                         start=True, stop=True)
                    epb = wide.tile([128, L], BF16, name="epb", tag="epb")
                    nc.scalar.activation(epb, ep, AF.Copy)
                    aenc = work.tile([128, 8], F32, name="aenc", tag="aenc")
                    scratch = wide.tile([128, L], BF16, name="scratch", tag="scratch")
                    for d in range(8):
                        nc.vector.scalar_tensor_tensor(
                            scratch, venc_sb[(d, xh)], 1.0, epb,
                            ALU.mult, ALU.mult,
                            accum_out=aenc[:, d:d + 1])
                    rc = work.tile([128, 1], F32, name="rc", tag="rc")
                    nc.vector.reciprocal(rc, at[:, xh * 9 + 8:xh * 9 + 9])
                    tsum = work.tile([128, 8], F32, name="tsum", tag="tsum")
                    nc.vector.tensor_tensor(out=tsum, in0=at[:, xh * 9:xh * 9 + 8],
                                            in1=aenc, op=ALU.add)
                    if first:
                        nc.vector.tensor_scalar(aoT[xh][:, 8 * h:8 * h + 8], tsum,
                                                rc[:, 0:1], None, ALU.mult)
                    else:
                        nc.vector.tensor_scalar(aoT[xh][:, 8 * h:8 * h + 8], tsum,
                                                rc[:, 0:1], 0.0, ALU.mult, ALU.max)
            if first:
                for xt in range(nxt):
                    nc.sync.dma_start(out=oHT[n, xt * 128:(xt + 1) * 128, :],
                                      in_=aoT[xt])
            else:
                aop = ps_tp.tile([64, 128], F32, name="tpp", tag="tpp")
                nc.tensor.matmul(aop, aoT[0], ident_sb, is_transpose=True,
                                 start=True, stop=True)
                ao_sb = work.tile([64, 128], F32, name="ao_sb", tag="ao_sb")
                nc.scalar.activation(ao_sb, aop, AF.Copy)
                c3p = ps_mm.tile([64, 128], F32, name="mmp", tag="mmp")
                nc.tensor.matmul(c3p, w3t_sb, ao_sb, start=True, stop=True)
                t2 = work.tile([64, 128], F32, name="t2", tag="t2")
                nc.vector.scalar_tensor_tensor(t2, c3p, c3s_sb[:, 0:1],
                                               res_sb[:, n * 128:(n + 1) * 128],
                                               ALU.mult, ALU.add)
                on = work.tile([64, 128], F32, name="on", tag="on")
                nc.scalar.activation(on, t2, AF.Relu, bias=c3b_sb[:, 0:1], scale=1.0)
                nc.sync.dma_start(out=outn[:, n * 128:(n + 1) * 128], in_=on)
    return nc


def _run(build_fn, in_maps):
    nc = bacc.Bacc()
    build_fn(nc)
    nc.finalize()
    import os
    tr = bool(int(os.environ.get("BASS_KERNEL_TRACE", "0")))
    r = run_bass_kernel_spmd(nc, in_maps, list(range(NCORES)), trace=tr)
    if tr and r.exec_time_ns is not None:
        print("launch exec_time_ns:", r.exec_time_ns)
    return r.results


def kernel(x, conv1_w, bn1, kqv_w_h, kqv_bn_h, logits_bn_h, rel_enc_h,
           kqv_w_w, kqv_bn_w, logits_bn_w, rel_enc_w, conv3_w, bn3):
    x = np.asarray(x, np.float32)
    B, C, H, W = x.shape  # 1, 64, 256, 128
    wsh = W // NCORES     # 16
    hsh = H // NCORES     # 32

    s1c, b1c = _bn_sb(np.asarray(bn1))
    s3c, b3c = _bn_sb(np.asarray(bn3))
    fh = _axial_folds(np.asarray(kqv_w_h), np.asarray(kqv_bn_h),
                      np.asarray(logits_bn_h))
    fw = _axial_folds(np.asarray(kqv_w_w), np.asarray(kqv_bn_w),
                      np.asarray(logits_bn_w))
    (wkqvt_h, ks4_h, kb4_h, qs4_h, qb4_h, vs_h, vb_h, s2_h, s31_h) = fh
    (wkqvt_w, ks4_w, kb4_w, qs4_w, qb4_w, vs_w, vb_w, s2_w, s31_w) = fw
    qe_h, ke_h, ve_h = _enc_arrays(np.asarray(rel_enc_h, np.float32), 256)
    qe_w, ke_w, ve_w = _enc_arrays(np.asarray(rel_enc_w, np.float32), 128)
    ident = np.eye(128, dtype=np.float32)

    shared1 = dict(
        w1t=np.ascontiguousarray(np.asarray(conv1_w, np.float32).T),
        c1s=s1c.reshape(64, 1), c1b=b1c.reshape(64, 1),
        wkqvt=wkqvt_h, ks4=ks4_h, kb4=kb4_h, qs4=qs4_h, qb4=qb4_h,
        vsc=vs_h, vbc=vb_h,
        qenc=qe_h, kenc=_bf16(ke_h), venc=_bf16(ve_h), ident=ident)
    in_maps1 = []
    for c in range(NCORES):
        xs = np.ascontiguousarray(
            x[0, :, :, c * wsh:(c + 1) * wsh]).reshape(64, 256 * wsh)
        m = dict(shared1)
        m["xin"] = xs
        in_maps1.append(m)
    res1 = _run(lambda nc: _build_axial(nc, 256, wsh, True, s2_h, s31_h), in_maps1)

    # gather oHT [n(w), 256(H), 64(c)] per core -> oH [64, 256, 128]
    oH = np.empty((64, 256, 128), np.float32)
    for c in range(NCORES):
        t = res1[c]["oHT"]  # [wsh, 256, 64]
        oH[:, :, c * wsh:(c + 1) * wsh] = t.transpose(2, 1, 0)

    shared2 = dict(
        w3t=np.ascontiguousarray(np.asarray(conv3_w, np.float32).T),
        c3s=s3c.reshape(64, 1), c3b=b3c.reshape(64, 1),
        wkqvt=wkqvt_w, ks4=ks4_w, kb4=kb4_w, qs4=qs4_w, qb4=qb4_w,
        vsc=vs_w, vbc=vb_w,
        qenc=qe_w, kenc=_bf16(ke_w), venc=_bf16(ve_w), ident=ident)
    in_maps2 = []
    for c in range(NCORES):
        sl = slice(c * hsh, (c + 1) * hsh)
        m = dict(shared2)
        m["xin"] = np.ascontiguousarray(oH[:, sl, :]).reshape(64, hsh * 128)
        m["res"] = np.ascontiguousarray(x[0, :, sl, :]).reshape(64, hsh * 128)
        in_maps2.append(m)
    res2 = _run(lambda nc: _build_axial(nc, 128, hsh, False, s2_w, s31_w), in_maps2)

    out = np.empty((1, 64, 256, 128), np.float32)
    for c in range(NCORES):
        out[0, :, c * hsh:(c + 1) * hsh, :] = res2[c]["outn"].reshape(64, hsh, 128)
    return out
